# revision 1
# baseline (speedup 1.0000x reference)
"""DGCN (GCNConv + self/change terms) on 8 Trainium2 NeuronCores.

Strategy (dst-sharded graph parallelism):
  - Output nodes (segment-sum destinations) are sharded across the 8 cores;
    each core owns a contiguous range of 64-node "dst tiles".
  - Host sorts edges (incl. self-loops) by (dst tile, src), pads each tile's
    edge list to multiples of 128, and builds per-core tables:
      ix16[128, 8*B] int16 gather indices (dma_gather layout: flat edge i of
                          a call at [i%16, i//16], replicated to the 8
                          16-partition Q7 groups)
      dstl[128, B] f32    local dst (0..63) within the tile
      nrm[128, B]  f32    edge weight dinv[src]*dinv[dst] (0 for padding)
  - Device, per dst tile t: dma_gather of x[src] rows (up to 512 rows per
    call), build a one-hot matrix oh[e, dst] = (iota == dstl_e) * nrm_e on
    the vector engine, and accumulate zT[d, dst] += msgs_e^T @ oh on the
    tensor engine in PSUM. This performs the whole normalized scatter-add
    as matmuls.
  - dma_gather indices are int16, so the gather table is split in two DRAM
    tensors: x_full rows [0, 32768) and x_hi rows [32768, n_pad); each
    tile's (src-sorted) edges are split lo/hi at block granularity.
  - Algebraic folding: out = h_neigh + x@W0 + (h_neigh - x)@Wt
        = (z @ Wc + bc) @ (I + Wt) + x @ (W0 - Wt)
        = z @ C + x @ B2 + b'
    with C = Wc @ (I + Wt), B2 = W0 - Wt, b' = bc @ (I + Wt), and z the
    normalized neighbor sum (incl. self loops) of raw x rows. So the x@Wc
    matmul is applied *after* aggregation on 64-row tiles (8x less matmul
    work than computing x@Wc for all N on every core) and each core needs
    only two small constant weights.
"""

import numpy as np

N_NODES = 50000
D = 128
N_CORES = 8
TILE_DST = 64  # dst nodes per tile (matmul free dim)
BLK = 128  # edges per matmul block (PE contraction dim)
HALF = 32768  # int16 index limit -> gather table split point
CALL_BLKS = 8  # max blocks (128 idxs each) per dma_gather call (1024-idx HW cap)
N_SWDGE_QUEUES = 4  # parallel SWDGE descriptor-generation queues

_NC_CACHE = {}


def _host_prep(x, edge_index, Wc, bc, W0, Wt, n_cores=N_CORES, tile_dst=TILE_DST):
    n, d = x.shape
    src = np.asarray(edge_index[0], dtype=np.int64)
    dst = np.asarray(edge_index[1], dtype=np.int64)

    # in-degree incl. self loop -> symmetric normalization factors
    deg = (np.bincount(dst, minlength=n) + 1).astype(np.float32)
    dinv = (1.0 / np.sqrt(deg)).astype(np.float32)

    loops = np.arange(n, dtype=np.int64)
    src_a = np.concatenate([src, loops])
    dst_a = np.concatenate([dst, loops])
    norm_a = (dinv[src_a] * dinv[dst_a]).astype(np.float32)

    tiles_total = -(-n // tile_dst)
    tiles_total = -(-tiles_total // n_cores) * n_cores
    tpc = tiles_total // n_cores
    n_pad = tiles_total * tile_dst
    rows_pc = tpc * tile_dst

    tile_of = dst_a // tile_dst
    order = np.lexsort((src_a, tile_of))
    src_s = src_a[order]
    dstl_s = (dst_a[order] - tile_of[order] * tile_dst).astype(np.float32)
    norm_s = norm_a[order]
    tile_s = tile_of[order]

    half = HALF if n_pad > HALF else n_pad

    counts = np.bincount(tile_s, minlength=tiles_total)
    tile_starts = np.zeros(tiles_total + 1, np.int64)
    tile_starts[1:] = np.cumsum(counts)
    # per (core, tile): lo/hi split position (edges sorted by src)
    lo_counts = np.zeros(tiles_total, np.int64)
    for g in range(tiles_total):
        s0, c = tile_starts[g], counts[g]
        lo_counts[g] = np.searchsorted(src_s[s0 : s0 + c], half)
    hi_counts = counts - lo_counts

    def nblk(c):
        return -(-c // BLK)

    NB_lo = np.zeros(tpc, np.int64)
    NB_hi = np.zeros(tpc, np.int64)
    for i in range(tpc):
        g = np.arange(n_cores) * tpc + i
        NB_lo[i] = nblk(lo_counts[g]).max()
        NB_hi[i] = nblk(hi_counts[g]).max()
        if NB_lo[i] + NB_hi[i] == 0:
            NB_lo[i] = 1
    F = NB_lo + NB_hi  # blocks per tile slot
    B = int(F.sum())
    off = np.zeros(tpc, np.int64)
    off[1:] = np.cumsum(F)[:-1]

    idx_flat = np.zeros((n_cores, B * BLK), np.int32)  # per-edge gather index
    dst_t = np.zeros((n_cores, BLK, B), np.float32)
    nrm_t = np.zeros((n_cores, BLK, B), np.float32)
    for k in range(n_cores):
        for i in range(tpc):
            g = k * tpc + i
            s0 = int(tile_starts[g])
            clo, chi = int(lo_counts[g]), int(hi_counts[g])
            o = int(off[i])
            # lo edges -> blocks [o, o+NB_lo), hi -> [o+NB_lo, o+F)
            for (cnt, base_blk, idx_shift, pos) in (
                (clo, o, 0, s0),
                (chi, o + int(NB_lo[i]), half, s0 + clo),
            ):
                if cnt == 0:
                    continue
                nb = nblk(cnt)
                cap = nb * BLK
                bi = np.zeros(cap, np.int32)
                bd = np.zeros(cap, np.float32)
                bn = np.zeros(cap, np.float32)
                bi[:cnt] = src_s[pos : pos + cnt] - idx_shift
                bd[:cnt] = dstl_s[pos : pos + cnt]
                bn[:cnt] = norm_s[pos : pos + cnt]
                e0 = base_blk * BLK
                idx_flat[k][e0 : e0 + cap] = bi
                cols = slice(base_blk, base_blk + nb)
                dst_t[k][:, cols] = bd.reshape(nb, BLK).T
                nrm_t[k][:, cols] = bn.reshape(nb, BLK).T

    # dma_gather int16 index tensor: within a call (<= CALL_BLKS blocks),
    # flat edge i of the call sits at [i % 16, w0 + i // 16], replicated
    # across the eight 16-partition groups. Because calls are aligned to
    # block boundaries and a block is 128 = 8*16 edges, the global wrap
    # below produces exactly the per-call layout for any block range.
    ix16 = np.zeros((n_cores, BLK, B * (BLK // 16)), np.int16)
    for k in range(n_cores):
        v = idx_flat[k].astype(np.int16).reshape(B * (BLK // 16), 16).T
        for c in range(8):
            ix16[k][16 * c : 16 * (c + 1), :] = v

    # fused weights
    Wc64 = np.asarray(Wc, np.float64)
    Wt64 = np.asarray(Wt, np.float64)
    W064 = np.asarray(W0, np.float64)
    bc64 = np.asarray(bc, np.float64)
    B1 = np.eye(d) + Wt64
    C = (Wc64 @ B1).astype(np.float32)
    B2 = (W064 - Wt64).astype(np.float32)
    bp = (bc64 @ B1).astype(np.float32)

    x_pad = np.zeros((n_pad, d), np.float32)
    x_pad[:n] = np.asarray(x, np.float32)

    consts = {
        "cw": C,
        "b2w": B2,
        "bpb": np.broadcast_to(bp, (tile_dst, d)).copy(),
        "iota": np.broadcast_to(
            np.arange(tile_dst, dtype=np.float32), (BLK, tile_dst)
        ).copy(),
        "ident": np.eye(BLK, dtype=np.float32),
    }
    x_hi_arr = x_pad[half:] if n_pad > half else np.zeros((1, d), np.float32)
    in_maps = []
    for k in range(n_cores):
        m = dict(consts)
        m["x_full"] = x_pad[:half]
        m["x_hi"] = x_hi_arr
        m["x_own"] = x_pad[k * rows_pc : (k + 1) * rows_pc].copy()
        m["ix16"] = ix16[k]
        m["dst_t"] = dst_t[k]
        m["nrm_t"] = nrm_t[k]
        in_maps.append(m)

    meta = dict(
        F=F,
        NB_lo=NB_lo,
        NB_hi=NB_hi,
        off=off,
        B=B,
        tpc=tpc,
        n_pad=n_pad,
        rows_pc=rows_pc,
        d=d,
        half=half,
        hi_rows=x_hi_arr.shape[0],
    )
    return in_maps, meta


def _build_nc(meta, n_cores=N_CORES, tile_dst=TILE_DST, repeat=1, ablate=()):
    """ablate: subset of {"gather","onehot","segmm","epilogue","xown"} to
    drop from the program (timing bisection only — output becomes wrong)."""
    import contextlib

    import concourse.bacc as bacc
    import concourse.mybir as mybir
    import concourse.tile as tile
    from concourse import library_config

    f32 = mybir.dt.float32
    i16 = mybir.dt.int16
    F, NB_lo, NB_hi, off = meta["F"], meta["NB_lo"], meta["NB_hi"], meta["off"]
    B, tpc = meta["B"], meta["tpc"]
    n_pad, rows_pc, d = meta["n_pad"], meta["rows_pc"], meta["d"]
    W16 = B * (BLK // 16)

    nc = bacc.Bacc(
        "TRN2",
        target_bir_lowering=False,
        debug=False,
        num_devices=n_cores,
        num_swdge_queues=N_SWDGE_QUEUES,
    )
    x_full = nc.declare_dram_parameter("x_full", [meta["half"], d], f32, isOutput=False)
    x_hi = nc.declare_dram_parameter("x_hi", [meta["hi_rows"], d], f32, isOutput=False)
    x_own = nc.declare_dram_parameter("x_own", [rows_pc, d], f32, isOutput=False)
    ix16 = nc.declare_dram_parameter("ix16", [BLK, W16], i16, isOutput=False)
    dst_t = nc.declare_dram_parameter("dst_t", [BLK, B], f32, isOutput=False)
    nrm_t = nc.declare_dram_parameter("nrm_t", [BLK, B], f32, isOutput=False)
    cw = nc.declare_dram_parameter("cw", [d, d], f32, isOutput=False)
    b2w = nc.declare_dram_parameter("b2w", [d, d], f32, isOutput=False)
    bpb = nc.declare_dram_parameter("bpb", [tile_dst, d], f32, isOutput=False)
    iota = nc.declare_dram_parameter("iota", [BLK, tile_dst], f32, isOutput=False)
    ident = nc.declare_dram_parameter("ident", [BLK, BLK], f32, isOutput=False)
    out = nc.declare_dram_parameter("out", [rows_pc, d], f32, isOutput=True)

    eq, mul, add = (
        mybir.AluOpType.is_equal,
        mybir.AluOpType.mult,
        mybir.AluOpType.add,
    )

    with tile.TileContext(nc) as tc:
        with (
            tc.tile_pool(name="const", bufs=1) as cpool,
            tc.tile_pool(name="tbl", bufs=1) as tpool,
            tc.tile_pool(name="gather", bufs=3) as gpool,
            tc.tile_pool(name="work", bufs=3) as wpool,
            tc.tile_pool(name="oh", bufs=4) as ohpool,
            tc.tile_pool(name="zps", bufs=2, space="PSUM") as zpool,
            tc.tile_pool(name="tps", bufs=2, space="PSUM") as tpspool,
            tc.tile_pool(name="ops", bufs=2, space="PSUM") as opool,
        ):
            nc.gpsimd.load_library(library_config.mlp)
            c_sb = cpool.tile([d, d], f32)
            nc.sync.dma_start(out=c_sb[:], in_=cw[:])
            b2_sb = cpool.tile([d, d], f32)
            nc.sync.dma_start(out=b2_sb[:], in_=b2w[:])
            bp_sb = cpool.tile([tile_dst, d], f32)
            nc.sync.dma_start(out=bp_sb[:], in_=bpb[:])
            io_sb = cpool.tile([BLK, tile_dst], f32)
            nc.sync.dma_start(out=io_sb[:], in_=iota[:])
            id_sb = cpool.tile([BLK, BLK], f32)
            nc.sync.dma_start(out=id_sb[:], in_=ident[:])
            ix_sb = tpool.tile([BLK, W16], i16)
            nc.sync.dma_start(out=ix_sb[:], in_=ix16[:])
            dl_sb = tpool.tile([BLK, B], f32)
            nc.sync.dma_start(out=dl_sb[:], in_=dst_t[:])
            nm_sb = tpool.tile([BLK, B], f32)
            nc.sync.dma_start(out=nm_sb[:], in_=nrm_t[:])

            _q = [0]  # round-robin SWDGE queue assignment for gathers
            # repeat>1 wraps the whole body in a device-side loop; used only
            # by the timing harness to amplify device time vs host overhead.
            rep_ctx = tc.For_i(0, repeat, 1) if repeat > 1 else contextlib.nullcontext()
            with rep_ctx:
                for i in range(tpc):
                    fi = int(F[i])
                    o = int(off[i])
                    g = gpool.tile([BLK, fi * d], f32, tag="g")
                    if "gather" not in ablate:
                        for (tbl, blk0, nb_total) in (
                            (x_full, 0, int(NB_lo[i])),
                            (x_hi, int(NB_lo[i]), int(NB_hi[i])),
                        ):
                            for c in range(0, nb_total, CALL_BLKS):
                                nb = min(CALL_BLKS, nb_total - c)
                                col = blk0 + c
                                nidx = nb * BLK
                                nc.gpsimd.dma_gather(
                                    out_ap=g[:, col * d : (col + nb) * d].rearrange(
                                        "p (n e) -> p n e", e=d
                                    ),
                                    in_ap=tbl[:],
                                    idxs_ap=ix_sb[
                                        :, (o + col) * 8 : (o + col + nb) * 8
                                    ],
                                    num_idxs=nidx,
                                    num_idxs_reg=nidx,
                                    elem_size=d,
                                    queue_num=_q[0] % N_SWDGE_QUEUES,
                                )
                                _q[0] += 1
                    if "xown" not in ablate:
                        xo = wpool.tile([tile_dst, d], f32, tag="xo")
                        nc.sync.dma_start(
                            out=xo[:], in_=x_own[i * tile_dst : (i + 1) * tile_dst, :]
                        )
                        xt_ps = tpspool.tile([d, tile_dst], f32)
                        nc.tensor.transpose(
                            out=xt_ps[:],
                            in_=xo[:],
                            identity=id_sb[:tile_dst, :tile_dst],
                        )
                        xt_sb = wpool.tile([d, tile_dst], f32, tag="xt")
                        nc.scalar.copy(out=xt_sb[:], in_=xt_ps[:])

                    z_ps = zpool.tile([d, tile_dst], f32)
                    for j in range(fi):
                        if "onehot" not in ablate:
                            oh = ohpool.tile([BLK, tile_dst], f32, tag="oh")
                            nc.vector.tensor_scalar(
                                out=oh[:],
                                in0=io_sb[:],
                                scalar1=dl_sb[:, o + j : o + j + 1],
                                scalar2=nm_sb[:, o + j : o + j + 1],
                                op0=eq,
                                op1=mul,
                            )
                            rhs_mm = oh[:]
                        else:
                            rhs_mm = io_sb[:]
                        if "segmm" not in ablate:
                            nc.tensor.matmul(
                                out=z_ps[:],
                                lhsT=g[:, j * d : (j + 1) * d],
                                rhs=rhs_mm,
                                start=(j == 0),
                                stop=(j == fi - 1),
                            )
                    if "epilogue" not in ablate:
                        if "segmm" in ablate:
                            nc.vector.memset(z_ps[:], 0.0)
                        z_sb = wpool.tile([d, tile_dst], f32, tag="z")
                        nc.scalar.copy(out=z_sb[:], in_=z_ps[:])

                        o_ps = opool.tile([tile_dst, d], f32)
                        nc.tensor.matmul(
                            out=o_ps[:],
                            lhsT=z_sb[:],
                            rhs=c_sb[:],
                            start=True,
                            stop=False,
                        )
                        nc.tensor.matmul(
                            out=o_ps[:],
                            lhsT=xt_sb[:],
                            rhs=b2_sb[:],
                            start=False,
                            stop=True,
                        )
                        o_sb = wpool.tile([tile_dst, d], f32, tag="o")
                        nc.vector.tensor_tensor(
                            out=o_sb[:], in0=o_ps[:], in1=bp_sb[:], op=add
                        )
                        nc.sync.dma_start(
                            out=out[i * tile_dst : (i + 1) * tile_dst, :], in_=o_sb[:]
                        )
    nc.compile()
    return nc


def _get_nc(meta, n_cores=N_CORES, tile_dst=TILE_DST):
    key = (tuple(int(f) for f in meta["F"]), tuple(int(f) for f in meta["NB_lo"]))
    if key not in _NC_CACHE:
        _NC_CACHE[key] = _build_nc(meta, n_cores=n_cores, tile_dst=tile_dst)
    return _NC_CACHE[key]


_LAST_RESULTS = None


def kernel(x, edge_index, Wc, bc, W0, Wt):
    global _LAST_RESULTS
    from concourse.bass_utils import run_bass_kernel_spmd

    x = np.asarray(x)
    n = x.shape[0]
    in_maps, meta = _host_prep(x, edge_index, Wc, bc, W0, Wt)
    nc = _get_nc(meta)
    res = run_bass_kernel_spmd(nc, in_maps, list(range(N_CORES)))
    _LAST_RESULTS = res
    outs = [res.results[k]["out"] for k in range(N_CORES)]
    return np.concatenate(outs, axis=0)[:n].astype(np.float32)



# revision 2
# speedup vs baseline: 1.2175x; 1.2175x over previous
"""DGCN (GCNConv + self/change terms) on 8 Trainium2 NeuronCores, v2.

Strategy (dst-sharded graph parallelism, cost-model-tuned):
  - 392 dst tiles of 128 nodes; each core owns 49 tiles (7 groups of 7),
    assigned by sorted-octet dealing so per-slot block counts (max over the
    8 cores) are tight. Host permutes tiles; output is un-permuted on host.
  - Algebra: out = z @ C + x @ B2 + b', with C = Wc(I+Wt), B2 = W0-Wt,
    b' = bc(I+Wt), and z the symmetric-normalized aggregation (incl. self
    loops) of raw x. dinv[src] is folded into the gather table
    (xs[v] = dinv[v]*x[v], bf16); dinv[dst] rides the one-hot.
  - Per 128-edge block: DVE builds oh[e, j] = (iota==dstl_e)*dinv_dst_e
    (bf16, f32 scalars); PE accumulates z_ps[d, dst] += g_blk^T @ oh.
  - Gathers are bf16 rows (elem_size=128), batched per tile-group into
    few large dma_gather calls (<=8192 idxs each) on 4 SWDGE queues.
    int16 idx limit -> lo/hi table split at row 32768.
  - Epilogue is orientation-flipped: o_ps[dout, dst] = C^T z + B2^T x_own,
    so the bias is per-partition and rides the ACT copy
    (activation(Identity, bias=b'_col)). Output written transposed
    ([128, rows_pc]) in one batched DMA per group; host transposes back.
"""

import numpy as np

N_NODES = 50000
D = 128
N_CORES = 8
TILE = 128          # dst nodes per tile
BLK = 128           # edges per matmul block (PE contraction dim)
HALF = 32768        # int16 index limit -> gather table split point
GROUP = 7           # tiles per group (shared gather calls / output DMA)
MAX_IDX_CALL = 1024  # max idxs per dma_gather call (Q7 ucode limit)
N_SWDGE_QUEUES = 4

N_TILES = 392       # ceil(50176 / 128), multiple of 8
TPC = N_TILES // N_CORES          # 49 tiles per core
N_GROUPS = TPC // GROUP           # 7 groups
N_PAD = N_TILES * TILE            # 50176
ROWS_PC = TPC * TILE              # 6272

_NC_CACHE = {}
_LAST_RESULTS = None


def _host_prep(x, edge_index, Wc, bc, W0, Wt):
    import ml_dtypes

    bf16 = ml_dtypes.bfloat16
    n, d = x.shape
    src = np.asarray(edge_index[0], dtype=np.int64)
    dst = np.asarray(edge_index[1], dtype=np.int64)

    deg = (np.bincount(dst, minlength=N_PAD) + 1).astype(np.float32)
    dinv = (1.0 / np.sqrt(deg)).astype(np.float32)

    loops = np.arange(n, dtype=np.int64)
    src_a = np.concatenate([src, loops])
    dst_a = np.concatenate([dst, loops])

    tile_g = dst_a // TILE
    order0 = np.lexsort((src_a, tile_g))
    src_s = src_a[order0]
    dst_s = dst_a[order0]
    tile_s = tile_g[order0]

    counts = np.bincount(tile_s, minlength=N_TILES)
    starts = np.zeros(N_TILES + 1, np.int64)
    starts[1:] = np.cumsum(counts)
    lo_counts = np.empty(N_TILES, np.int64)
    for g in range(N_TILES):
        s0, c = starts[g], counts[g]
        lo_counts[g] = np.searchsorted(src_s[s0:s0 + c], HALF)
    hi_counts = counts - lo_counts

    # balanced octet assignment: sort tiles by (lo blocks, hi count), deal
    # sorted octet i across the 8 cores as slot i.
    nbl_t = -(-lo_counts // BLK)
    order_t = np.lexsort((hi_counts, lo_counts, nbl_t))
    assign = order_t.reshape(TPC, N_CORES)          # [slot, core] -> tile
    F_lo = np.zeros(TPC, np.int64)
    F_hi = np.zeros(TPC, np.int64)
    for i in range(TPC):
        octet = assign[i]
        F_lo[i] = -(-lo_counts[octet].max() // BLK)
        F_hi[i] = -(-hi_counts[octet].max() // BLK)
        if F_lo[i] + F_hi[i] == 0:
            F_lo[i] = 1

    # group-level block layout: per group q, cols = [lo(slot 7q..7q+6) then
    # hi(...)]; global block col = group offset + local col.
    grp_nb = np.zeros(N_GROUPS, np.int64)
    grp_off = np.zeros(N_GROUPS + 1, np.int64)
    slot_lo_col = np.zeros(TPC, np.int64)   # global block col of slot's lo
    slot_hi_col = np.zeros(TPC, np.int64)
    callplan = []                           # per group: list of (is_hi, local_b0, nb)
    for q in range(N_GROUPS):
        sl = range(q * GROUP, (q + 1) * GROUP)
        nb_lo = int(sum(F_lo[i] for i in sl))
        nb_hi = int(sum(F_hi[i] for i in sl))
        grp_nb[q] = nb_lo + nb_hi
        grp_off[q + 1] = grp_off[q] + grp_nb[q]
        c = 0
        for i in sl:
            slot_lo_col[i] = grp_off[q] + c
            c += F_lo[i]
        for i in sl:
            slot_hi_col[i] = grp_off[q] + c
            c += F_hi[i]
        calls = []
        max_blk = MAX_IDX_CALL // BLK
        for is_hi, b0, nb in ((0, 0, nb_lo), (1, nb_lo, nb_hi)):
            while nb > 0:
                take = min(nb, max_blk)
                calls.append((is_hi, b0, take))
                b0 += take
                nb -= take
        callplan.append(calls)
    NB = int(grp_off[N_GROUPS])

    # per-core tables
    idx_flat = np.zeros((N_CORES, NB * BLK), np.int32)
    dl_t = np.full((N_CORES, BLK, NB), 1000.0, np.float32)
    dd_t = np.zeros((N_CORES, BLK, NB), np.float32)
    for k in range(N_CORES):
        for i in range(TPC):
            g = assign[i, k]
            s0 = int(starts[g])
            clo, chi = int(lo_counts[g]), int(hi_counts[g])
            base = g * TILE
            for cnt, col0, shift, pos, fcap in (
                (clo, int(slot_lo_col[i]), 0, s0, int(F_lo[i])),
                (chi, int(slot_hi_col[i]), HALF, s0 + clo, int(F_hi[i])),
            ):
                if cnt == 0:
                    continue
                cap = fcap * BLK
                e0 = col0 * BLK
                idx_flat[k, e0:e0 + cnt] = src_s[pos:pos + cnt] - shift
                bd = np.full(cap, 1000.0, np.float32)
                bd[:cnt] = (dst_s[pos:pos + cnt] - base).astype(np.float32)
                bn = np.zeros(cap, np.float32)
                bn[:cnt] = dinv[dst_s[pos:pos + cnt]]
                dl_t[k][:, col0:col0 + fcap] = bd.reshape(fcap, BLK).T
                dd_t[k][:, col0:col0 + fcap] = bn.reshape(fcap, BLK).T

    # dma_gather idx layout: flat edge j of a block-aligned call sits at
    # [j % 16, j // 16], replicated across the eight 16-partition groups.
    W16 = NB * (BLK // 16)
    ix16 = np.empty((N_CORES, BLK, W16), np.int16)
    for k in range(N_CORES):
        v = idx_flat[k].astype(np.int16).reshape(W16, 16).T
        ix16[k] = np.tile(v, (8, 1))

    # fused weights (float64 for exactness, then cast)
    Wc64 = np.asarray(Wc, np.float64)
    Wt64 = np.asarray(Wt, np.float64)
    W064 = np.asarray(W0, np.float64)
    bc64 = np.asarray(bc, np.float64)
    B1 = np.eye(d) + Wt64
    C = (Wc64 @ B1).astype(np.float32)
    B2 = (W064 - Wt64).astype(np.float32)
    bp = (bc64 @ B1).astype(np.float32)

    x_pad = np.zeros((N_PAD, d), np.float32)
    x_pad[:n] = np.asarray(x, np.float32)
    xs = (x_pad * dinv[:, None]).astype(bf16)

    iota = np.broadcast_to(np.arange(TILE, dtype=np.float32),
                           (BLK, TILE)).astype(bf16)

    # per-core owned nodes (slot order) for x_own_T and output unshard
    node_ids = np.empty((N_CORES, ROWS_PC), np.int64)
    for k in range(N_CORES):
        for i in range(TPC):
            g = assign[i, k]
            node_ids[k, i * TILE:(i + 1) * TILE] = np.arange(
                g * TILE, (g + 1) * TILE)

    in_maps = []
    for k in range(N_CORES):
        m = {
            "x_lo": xs[:HALF],
            "x_hi": xs[HALF:],
            "xT": np.ascontiguousarray(x_pad[node_ids[k]].T.astype(bf16)),
            "ix16": ix16[k],
            "dl": dl_t[k],
            "dd": dd_t[k],
            "cw": C.astype(bf16),
            "b2w": B2.astype(bf16),
            "bpc": bp.reshape(d, 1),
            "iota": np.ascontiguousarray(iota),
        }
        in_maps.append(m)

    meta = dict(
        F_lo=F_lo, F_hi=F_hi, slot_lo_col=slot_lo_col,
        slot_hi_col=slot_hi_col, grp_nb=grp_nb, grp_off=grp_off,
        callplan=callplan, NB=NB, W16=W16, hi_rows=N_PAD - HALF,
        node_ids=node_ids,
    )
    return in_maps, meta


def _build_nc(meta, ablate=()):
    import concourse.bacc as bacc
    import concourse.mybir as mybir
    import concourse.tile as tile
    from concourse import library_config

    f32 = mybir.dt.float32
    bf16 = mybir.dt.bfloat16
    i16 = mybir.dt.int16
    eq, mul = mybir.AluOpType.is_equal, mybir.AluOpType.mult
    ident = mybir.ActivationFunctionType.Identity

    F_lo, F_hi = meta["F_lo"], meta["F_hi"]
    slot_lo_col, slot_hi_col = meta["slot_lo_col"], meta["slot_hi_col"]
    grp_nb, grp_off = meta["grp_nb"], meta["grp_off"]
    callplan, NB, W16 = meta["callplan"], meta["NB"], meta["W16"]

    nc = bacc.Bacc(
        "TRN2",
        target_bir_lowering=False,
        debug=False,
        num_devices=N_CORES,
        num_swdge_queues=N_SWDGE_QUEUES,
    )
    x_lo = nc.declare_dram_parameter("x_lo", [HALF, D], bf16, isOutput=False)
    x_hi = nc.declare_dram_parameter("x_hi", [meta["hi_rows"], D], bf16,
                                     isOutput=False)
    xT = nc.declare_dram_parameter("xT", [D, ROWS_PC], bf16, isOutput=False)
    ix16 = nc.declare_dram_parameter("ix16", [BLK, W16], i16, isOutput=False)
    dl = nc.declare_dram_parameter("dl", [BLK, NB], f32, isOutput=False)
    dd = nc.declare_dram_parameter("dd", [BLK, NB], f32, isOutput=False)
    cw = nc.declare_dram_parameter("cw", [D, D], bf16, isOutput=False)
    b2w = nc.declare_dram_parameter("b2w", [D, D], bf16, isOutput=False)
    bpc = nc.declare_dram_parameter("bpc", [D, 1], f32, isOutput=False)
    iota = nc.declare_dram_parameter("iota", [BLK, TILE], bf16, isOutput=False)
    out = nc.declare_dram_parameter("out", [D, ROWS_PC], f32, isOutput=True)

    with tile.TileContext(nc) as tc:
        with (
            tc.tile_pool(name="const", bufs=1) as cpool,
            tc.tile_pool(name="tbl", bufs=1) as tpool,
            tc.tile_pool(name="gather", bufs=2) as gpool,
            tc.tile_pool(name="oh", bufs=4) as ohpool,
            tc.tile_pool(name="z", bufs=3) as zsbpool,
            tc.tile_pool(name="og", bufs=2) as ogpool,
            tc.tile_pool(name="zps", bufs=2, space="PSUM") as zpool,
            tc.tile_pool(name="ops", bufs=2, space="PSUM") as opool,
        ):
            nc.gpsimd.load_library(library_config.mlp)
            c_sb = cpool.tile([D, D], bf16)
            nc.sync.dma_start(out=c_sb[:], in_=cw[:])
            b2_sb = cpool.tile([D, D], bf16)
            nc.sync.dma_start(out=b2_sb[:], in_=b2w[:])
            bp_sb = cpool.tile([D, 1], f32)
            nc.sync.dma_start(out=bp_sb[:], in_=bpc[:])
            io_sb = cpool.tile([BLK, TILE], bf16)
            nc.sync.dma_start(out=io_sb[:], in_=iota[:])
            xT_sb = cpool.tile([D, ROWS_PC], bf16)
            nc.sync.dma_start(out=xT_sb[:], in_=xT[:])
            ix_sb = tpool.tile([BLK, W16], i16)
            nc.sync.dma_start(out=ix_sb[:], in_=ix16[:])
            dl_sb = tpool.tile([BLK, NB], f32)
            nc.sync.dma_start(out=dl_sb[:], in_=dl[:])
            dd_sb = tpool.tile([BLK, NB], f32)
            nc.sync.dma_start(out=dd_sb[:], in_=dd[:])

            qrr = [0]
            for q in range(N_GROUPS):
                gb0 = int(grp_off[q])
                gnb = int(grp_nb[q])
                g_sb = gpool.tile([BLK, gnb * D], bf16, tag="g")
                if "gather" not in ablate:
                    for (is_hi, b0, nb) in callplan[q]:
                        tbl = x_hi if is_hi else x_lo
                        nidx = nb * BLK
                        nc.gpsimd.dma_gather(
                            out_ap=g_sb[:, b0 * D:(b0 + nb) * D].rearrange(
                                "p (n e) -> p n e", e=D),
                            in_ap=tbl[:],
                            idxs_ap=ix_sb[:, (gb0 + b0) * 8:(gb0 + b0 + nb) * 8],
                            num_idxs=nidx,
                            num_idxs_reg=nidx,
                            elem_size=D,
                            queue_num=qrr[0] % N_SWDGE_QUEUES,
                        )
                        qrr[0] += 1
                og_sb = ogpool.tile([D, GROUP * TILE], f32, tag="og")
                for s in range(GROUP):
                    i = q * GROUP + s
                    nblk = int(F_lo[i] + F_hi[i])
                    z_ps = zpool.tile([D, TILE], f32)
                    jj = 0
                    for (col0, fcnt) in ((int(slot_lo_col[i]), int(F_lo[i])),
                                         (int(slot_hi_col[i]), int(F_hi[i]))):
                        for b in range(col0, col0 + fcnt):
                            lb = b - gb0
                            if "onehot" not in ablate:
                                oh = ohpool.tile([BLK, TILE], bf16, tag="oh")
                                nc.vector.tensor_scalar(
                                    out=oh[:], in0=io_sb[:],
                                    scalar1=dl_sb[:, b:b + 1],
                                    scalar2=dd_sb[:, b:b + 1],
                                    op0=eq, op1=mul,
                                )
                                rhs = oh[:]
                            else:
                                rhs = io_sb[:]
                            if "segmm" not in ablate:
                                nc.tensor.matmul(
                                    out=z_ps[:],
                                    lhsT=g_sb[:, lb * D:(lb + 1) * D],
                                    rhs=rhs,
                                    start=(jj == 0), stop=(jj == nblk - 1),
                                )
                            jj += 1
                    if "epilogue" in ablate:
                        continue
                    if "segmm" in ablate:
                        nc.vector.memset(z_ps[:], 0.0)
                    z_sb = zsbpool.tile([D, TILE], bf16, tag="z")
                    nc.scalar.copy(out=z_sb[:], in_=z_ps[:])
                    o_ps = opool.tile([D, TILE], f32)
                    nc.tensor.matmul(out=o_ps[:], lhsT=c_sb[:], rhs=z_sb[:],
                                     start=True, stop=False)
                    nc.tensor.matmul(out=o_ps[:], lhsT=b2_sb[:],
                                     rhs=xT_sb[:, i * TILE:(i + 1) * TILE],
                                     start=False, stop=True)
                    nc.scalar.activation(
                        out=og_sb[:, s * TILE:(s + 1) * TILE], in_=o_ps[:],
                        func=ident, bias=bp_sb[:, 0:1])
                if "epilogue" not in ablate:
                    nc.sync.dma_start(
                        out=out[:, q * GROUP * TILE:(q + 1) * GROUP * TILE],
                        in_=og_sb[:])
    nc.compile()
    return nc


def _meta_key(meta):
    return (
        tuple(int(v) for v in meta["F_lo"]),
        tuple(int(v) for v in meta["F_hi"]),
    )


def _get_nc(meta):
    key = _meta_key(meta)
    if key not in _NC_CACHE:
        _NC_CACHE[key] = _build_nc(meta)
    return _NC_CACHE[key]


def kernel(x, edge_index, Wc, bc, W0, Wt):
    global _LAST_RESULTS
    from concourse.bass_utils import run_bass_kernel_spmd

    x = np.asarray(x)
    n = x.shape[0]
    in_maps, meta = _host_prep(x, edge_index, Wc, bc, W0, Wt)
    nc = _get_nc(meta)
    res = run_bass_kernel_spmd(nc, in_maps, list(range(N_CORES)))
    _LAST_RESULTS = res
    out_full = np.empty((N_PAD, D), np.float32)
    for k in range(N_CORES):
        out_full[meta["node_ids"][k]] = res.results[k]["out"].T
    return out_full[:n].astype(np.float32)


# revision 3
# speedup vs baseline: 1.2896x; 1.0592x over previous
"""DGCN on 8 Trainium2 NeuronCores, v3: shared-boundary max-profile packing.

Differences vs v2:
  - Within a (group, half) gather region, slots are packed back-to-back at
    their max-over-cores edge counts (no per-slot ceil-to-128); only each
    region is block-aligned. Blocks spanning a slot boundary get one
    matmul+one-hot per covered slot (per-view dl columns).
  - Octets balanced by total edge count (primary) then lo count.
  - Flexible group sizes, ending with a tiny group so the post-gather
    pipeline tail is short.
"""

import numpy as np

N_NODES = 50000
D = 128
N_CORES = 8
TILE = 128
BLK = 128
HALF = 32768
MAX_IDX_CALL = 1024  # Q7 ucode limit per dma_gather call
N_SWDGE_QUEUES = 4

N_TILES = 392
TPC = N_TILES // N_CORES          # 49
GROUP_SIZES = [10, 10, 10, 10, 8, 1]  # slots per group; tiny last = short tail
N_PAD = N_TILES * TILE
ROWS_PC = TPC * TILE

_NC_CACHE = {}
_LAST_RESULTS = None


def _host_prep(x, edge_index, Wc, bc, W0, Wt):
    import ml_dtypes

    bf16 = ml_dtypes.bfloat16
    n, d = x.shape
    src = np.asarray(edge_index[0], dtype=np.int64)
    dst = np.asarray(edge_index[1], dtype=np.int64)

    deg = (np.bincount(dst, minlength=N_PAD) + 1).astype(np.float32)
    dinv = (1.0 / np.sqrt(deg)).astype(np.float32)

    loops = np.arange(n, dtype=np.int64)
    src_a = np.concatenate([src, loops])
    dst_a = np.concatenate([dst, loops])

    tile_g = dst_a // TILE
    order0 = np.lexsort((src_a, tile_g))
    src_s = src_a[order0]
    dst_s = dst_a[order0]
    tile_s = tile_g[order0]

    counts = np.bincount(tile_s, minlength=N_TILES)
    starts = np.zeros(N_TILES + 1, np.int64)
    starts[1:] = np.cumsum(counts)
    lo_counts = np.empty(N_TILES, np.int64)
    for g in range(N_TILES):
        s0, c = starts[g], counts[g]
        lo_counts[g] = np.searchsorted(src_s[s0:s0 + c], HALF)
    hi_counts = counts - lo_counts

    # octet balancing: sort tiles by (lo, hi) DESCENDING and deal octet i
    # across the 8 cores as slot i; biggest slots first so the final (tiny)
    # group holds the smallest tiles. Then a pairwise same-core swap
    # hill-climb tightens sum(max lo + max hi) over octets.
    order_t = np.lexsort((hi_counts, lo_counts))[::-1]
    assign = order_t.reshape(TPC, N_CORES).copy()   # [slot, core] -> tile
    for _sweep in range(3):
        improved = 0
        alo = lo_counts[assign]
        ahi = hi_counts[assign]
        for i in range(TPC):
            for j in range(i + 1, TPC):
                base_cost = (alo[i].max() + ahi[i].max()
                             + alo[j].max() + ahi[j].max())
                for k in range(N_CORES):
                    alo[i, k], alo[j, k] = alo[j, k], alo[i, k]
                    ahi[i, k], ahi[j, k] = ahi[j, k], ahi[i, k]
                    new_cost = (alo[i].max() + ahi[i].max()
                                + alo[j].max() + ahi[j].max())
                    if new_cost < base_cost:
                        assign[i, k], assign[j, k] = assign[j, k], assign[i, k]
                        base_cost = new_cost
                        improved += 1
                    else:
                        alo[i, k], alo[j, k] = alo[j, k], alo[i, k]
                        ahi[i, k], ahi[j, k] = ahi[j, k], ahi[i, k]
        if improved == 0:
            break
    L_lo = lo_counts[assign].max(axis=1)            # [slot] max-profile lens
    L_hi = hi_counts[assign].max(axis=1)

    assert sum(GROUP_SIZES) == TPC
    grp_slots = []
    s0_ = 0
    for gs in GROUP_SIZES:
        grp_slots.append(list(range(s0_, s0_ + gs)))
        s0_ += gs

    # region/block/view layout
    NBASE = 0                    # running global block count
    callplan = []                # per group: (is_hi, local_b0, nb_blocks)
    grp_nb = []                  # blocks per group
    grp_off = []                 # global block offset per group
    slot_views_h = [([], []) for _ in range(TPC)]  # per half: (global_block, dl_col)
    # per-slot placement info for table building:
    place = {}                   # (slot, half) -> (region_pos, region_glb_b0)
    nview = 0
    for q, sl in enumerate(grp_slots):
        grp_off.append(NBASE)
        calls = []
        gb = 0                   # group-local block counter
        for h, L in ((0, L_lo), (1, L_hi)):
            pos = 0
            covered = []         # (slot, p0, p1) in region slot-positions
            for i in sl:
                li = int(L[i])
                place[(i, h)] = (pos, NBASE + gb)
                if li > 0:
                    covered.append((i, pos, pos + li))
                pos += li
            rblocks = -(-pos // BLK)
            # views: block b covers slot i iff ranges overlap
            for b in range(rblocks):
                lo_p, hi_p = b * BLK, (b + 1) * BLK
                for (i, p0, p1) in covered:
                    if p0 < hi_p and p1 > lo_p:
                        slot_views_h[i][h].append((NBASE + gb + b, nview))
                        nview += 1
            # calls (block-aligned, <=8 blocks each)
            b0 = gb
            nb = rblocks
            while nb > 0:
                take = min(nb, MAX_IDX_CALL // BLK)
                calls.append((h, b0, take))
                b0 += take
                nb -= take
            gb += rblocks
        grp_nb.append(gb)
        NBASE += gb
        callplan.append(calls)
    NB = NBASE
    NVIEWS = nview

    # tables
    idx_flat = np.zeros((N_CORES, NB * BLK), np.int32)
    dd_t = np.zeros((N_CORES, BLK, NB), np.float32)
    dl_t = np.full((N_CORES, BLK, NVIEWS), 1000.0, np.float32)
    for k in range(N_CORES):
        for i in range(TPC):
            g = assign[i, k]
            s0 = int(starts[g])
            clo, chi = int(lo_counts[g]), int(hi_counts[g])
            base = g * TILE
            for h, cnt, shift, pos0 in ((0, clo, 0, s0), (1, chi, HALF, s0 + clo)):
                if cnt == 0:
                    continue
                rpos, rgb0 = place[(i, h)]
                e0 = rgb0 * BLK + rpos
                idx_flat[k, e0:e0 + cnt] = src_s[pos0:pos0 + cnt] - shift
                # dd_t is [BLK, NB] with flat pos j -> [j%BLK, j//BLK]
                jj = np.arange(e0, e0 + cnt)
                dd_t[k][jj % BLK, jj // BLK] = dinv[dst_s[pos0:pos0 + cnt]]
        # dl per view
        for i in range(TPC):
            g = assign[i, k]
            s0 = int(starts[g])
            clo, chi = int(lo_counts[g]), int(hi_counts[g])
            base = g * TILE
            for (gb, vcol) in slot_views_h[i][0] + slot_views_h[i][1]:
                # which half does this view belong to?
                # find via placement: check lo then hi range
                done = False
                for h, cnt, pos0 in ((0, clo, s0), (1, chi, s0 + clo)):
                    rpos, rgb0 = place.get((i, h), (None, None))
                    if rpos is None:
                        continue
                    li = int((L_lo if h == 0 else L_hi)[i])
                    b_lo = rgb0 * BLK + rpos          # abs slot-pos of slot start
                    b_hi = b_lo + li
                    blk_lo, blk_hi = gb * BLK, (gb + 1) * BLK
                    if b_lo < blk_hi and b_hi > blk_lo:
                        # positions of this block within the slot's edge list
                        p_start = max(b_lo, blk_lo)
                        p_end = min(b_hi, blk_hi)
                        # rows within the block
                        r0, r1 = p_start - blk_lo, p_end - blk_lo
                        # edge offsets within slot's count
                        eo0 = p_start - b_lo
                        m = min(cnt - eo0, r1 - r0)
                        if m > 0:
                            dl_t[k][r0:r0 + m, vcol] = (
                                dst_s[pos0 + eo0:pos0 + eo0 + m] - base
                            ).astype(np.float32)
                        done = True
                if not done:
                    pass

    W16 = NB * (BLK // 16)
    ix16 = np.empty((N_CORES, BLK, W16), np.int16)
    for k in range(N_CORES):
        v = idx_flat[k].astype(np.int16).reshape(W16, 16).T
        ix16[k] = np.tile(v, (8, 1))

    Wc64 = np.asarray(Wc, np.float64)
    Wt64 = np.asarray(Wt, np.float64)
    W064 = np.asarray(W0, np.float64)
    bc64 = np.asarray(bc, np.float64)
    B1 = np.eye(d) + Wt64
    C = (Wc64 @ B1).astype(np.float32)
    B2 = (W064 - Wt64).astype(np.float32)
    bp = (bc64 @ B1).astype(np.float32)

    x_pad = np.zeros((N_PAD, d), np.float32)
    x_pad[:n] = np.asarray(x, np.float32)
    xs = (x_pad * dinv[:, None]).astype(bf16)

    iota = np.broadcast_to(np.arange(TILE, dtype=np.float32),
                           (BLK, TILE)).astype(bf16)

    node_ids = np.empty((N_CORES, ROWS_PC), np.int64)
    for k in range(N_CORES):
        for i in range(TPC):
            g = assign[i, k]
            node_ids[k, i * TILE:(i + 1) * TILE] = np.arange(
                g * TILE, (g + 1) * TILE)

    in_maps = []
    for k in range(N_CORES):
        m = {
            "x_lo": xs[:HALF],
            "x_hi": xs[HALF:],
            "xT": np.ascontiguousarray(x_pad[node_ids[k]].T.astype(bf16)),
            "ix16": ix16[k],
            "dl": dl_t[k].astype(bf16),
            "dd": dd_t[k].astype(bf16),
            "cw": C.astype(bf16),
            "b2w": B2.astype(bf16),
            "bpc": bp.reshape(d, 1),
            "iota": np.ascontiguousarray(iota),
        }
        in_maps.append(m)

    meta = dict(
        grp_slots=grp_slots, grp_nb=grp_nb, grp_off=grp_off,
        callplan=callplan, slot_views=slot_views_h, NB=NB, NVIEWS=NVIEWS,
        W16=W16, hi_rows=N_PAD - HALF, node_ids=node_ids,
    )
    return in_maps, meta


def _build_nc(meta, ablate=()):
    import concourse.bacc as bacc
    import concourse.mybir as mybir
    import concourse.tile as tile
    from concourse import library_config

    f32 = mybir.dt.float32
    bf16 = mybir.dt.bfloat16
    i16 = mybir.dt.int16
    eq, mul = mybir.AluOpType.is_equal, mybir.AluOpType.mult
    ident = mybir.ActivationFunctionType.Identity

    grp_slots, grp_nb, grp_off = meta["grp_slots"], meta["grp_nb"], meta["grp_off"]
    callplan, slot_views = meta["callplan"], meta["slot_views"]
    NB, NVIEWS, W16 = meta["NB"], meta["NVIEWS"], meta["W16"]

    nc = bacc.Bacc(
        "TRN2",
        target_bir_lowering=False,
        debug=False,
        num_devices=N_CORES,
        num_swdge_queues=N_SWDGE_QUEUES,
    )
    x_lo = nc.declare_dram_parameter("x_lo", [HALF, D], bf16, isOutput=False)
    x_hi = nc.declare_dram_parameter("x_hi", [meta["hi_rows"], D], bf16,
                                     isOutput=False)
    xT = nc.declare_dram_parameter("xT", [D, ROWS_PC], bf16, isOutput=False)
    ix16 = nc.declare_dram_parameter("ix16", [BLK, W16], i16, isOutput=False)
    dl = nc.declare_dram_parameter("dl", [BLK, NVIEWS], bf16, isOutput=False)
    dd = nc.declare_dram_parameter("dd", [BLK, NB], bf16, isOutput=False)
    cw = nc.declare_dram_parameter("cw", [D, D], bf16, isOutput=False)
    b2w = nc.declare_dram_parameter("b2w", [D, D], bf16, isOutput=False)
    bpc = nc.declare_dram_parameter("bpc", [D, 1], f32, isOutput=False)
    iota = nc.declare_dram_parameter("iota", [BLK, TILE], bf16, isOutput=False)
    out = nc.declare_dram_parameter("out", [D, ROWS_PC], bf16, isOutput=True)

    with tile.TileContext(nc) as tc:
        with (
            tc.tile_pool(name="const", bufs=1) as cpool,
            tc.tile_pool(name="tbl", bufs=1) as tpool,
            tc.tile_pool(name="gather", bufs=2) as gpool,
            tc.tile_pool(name="oh", bufs=4) as ohpool,
            tc.tile_pool(name="z", bufs=24) as zsbpool,
            tc.tile_pool(name="og", bufs=2) as ogpool,
            tc.tile_pool(name="zps", bufs=6, space="PSUM") as zpool,
            tc.tile_pool(name="ops", bufs=2, space="PSUM") as opool,
        ):
            nc.gpsimd.load_library(library_config.mlp)
            c_sb = cpool.tile([D, D], bf16)
            nc.sync.dma_start(out=c_sb[:], in_=cw[:])
            b2_sb = cpool.tile([D, D], bf16)
            nc.sync.dma_start(out=b2_sb[:], in_=b2w[:])
            bp_sb = cpool.tile([D, 1], f32)
            nc.sync.dma_start(out=bp_sb[:], in_=bpc[:])
            io_sb = cpool.tile([BLK, TILE], bf16)
            nc.sync.dma_start(out=io_sb[:], in_=iota[:])
            xT_sb = cpool.tile([D, ROWS_PC], bf16)
            ix_sb = tpool.tile([BLK, W16], i16)
            sl0 = min(16, int(grp_nb[0])) * 8
            nc.sync.dma_start(out=ix_sb[:, :sl0], in_=ix16[:, :sl0])
            dl_lb = tpool.tile([BLK, NVIEWS], bf16)
            nc.sync.dma_start(out=dl_lb[:], in_=dl[:])
            dd_lb = tpool.tile([BLK, NB], bf16)
            nc.sync.dma_start(out=dd_lb[:], in_=dd[:])
            dl_sb = tpool.tile([BLK, NVIEWS], f32)
            nc.vector.tensor_copy(out=dl_sb[:], in_=dl_lb[:])
            dd_sb = tpool.tile([BLK, NB], f32)
            nc.vector.tensor_copy(out=dd_sb[:], in_=dd_lb[:])
            if int(grp_nb[0]) * 8 > sl0:
                nc.sync.dma_start(out=ix_sb[:, sl0:int(grp_nb[0]) * 8],
                                  in_=ix16[:, sl0:int(grp_nb[0]) * 8])
            for q in range(1, len(grp_slots)):
                a, b = int(grp_off[q]) * 8, (int(grp_off[q]) + int(grp_nb[q])) * 8
                nc.sync.dma_start(out=ix_sb[:, a:b], in_=ix16[:, a:b])
            nc.sync.dma_start(out=xT_sb[:], in_=xT[:])

            qrr = [0]
            for q, sl in enumerate(grp_slots):
                gb0 = int(grp_off[q])
                gnb = int(grp_nb[q])
                g_sb = gpool.tile([BLK, gnb * D], bf16, tag="g")
                if "gather" not in ablate:
                    for (is_hi, b0, nb) in callplan[q]:
                        tbl = x_hi if is_hi else x_lo
                        nidx = nb * BLK
                        nc.gpsimd.dma_gather(
                            out_ap=g_sb[:, b0 * D:(b0 + nb) * D].rearrange(
                                "p (n e) -> p n e", e=D),
                            in_ap=tbl[:],
                            idxs_ap=ix_sb[:, (gb0 + b0) * 8:(gb0 + b0 + nb) * 8],
                            num_idxs=nidx,
                            num_idxs_reg=nidx,
                            elem_size=D,
                            queue_num=qrr[0] % N_SWDGE_QUEUES,
                        )
                        qrr[0] += 1
                og_sb = ogpool.tile([D, len(sl) * TILE], bf16, tag="og")
                zparts = {}
                for h in (0, 1):
                    for i in sl:
                        views = slot_views[i][h]
                        if not views or "segmm" in ablate:
                            continue
                        nv = len(views)
                        z_ps = zpool.tile([D, TILE], f32)
                        for jj, (gb, vcol) in enumerate(views):
                            lb = gb - gb0
                            if "onehot" not in ablate:
                                oh = ohpool.tile([BLK, TILE], bf16, tag="oh")
                                nc.vector.tensor_scalar(
                                    out=oh[:], in0=io_sb[:],
                                    scalar1=dl_sb[:, vcol:vcol + 1],
                                    scalar2=dd_sb[:, gb:gb + 1],
                                    op0=eq, op1=mul,
                                )
                                rhs = oh[:]
                            else:
                                rhs = io_sb[:]
                            nc.tensor.matmul(
                                out=z_ps[:],
                                lhsT=g_sb[:, lb * D:(lb + 1) * D],
                                rhs=rhs,
                                start=(jj == 0), stop=(jj == nv - 1),
                            )
                        z_sb = zsbpool.tile([D, TILE], bf16, tag="z")
                        nc.scalar.copy(out=z_sb[:], in_=z_ps[:])
                        zparts.setdefault(i, []).append(z_sb)
                if "epilogue" not in ablate:
                    for s, i in enumerate(sl):
                        o_ps = opool.tile([D, TILE], f32)
                        parts = zparts.get(i, [])
                        for z_sb in parts:
                            nc.tensor.matmul(out=o_ps[:], lhsT=c_sb[:],
                                             rhs=z_sb[:],
                                             start=(z_sb is parts[0]),
                                             stop=False)
                        nc.tensor.matmul(out=o_ps[:], lhsT=b2_sb[:],
                                         rhs=xT_sb[:, i * TILE:(i + 1) * TILE],
                                         start=(not parts), stop=True)
                        nc.scalar.activation(
                            out=og_sb[:, s * TILE:(s + 1) * TILE], in_=o_ps[:],
                            func=ident, bias=bp_sb[:, 0:1])
                if "epilogue" not in ablate:
                    col0 = sl[0] * TILE
                    nc.sync.dma_start(
                        out=out[:, col0:col0 + len(sl) * TILE], in_=og_sb[:])
    nc.compile()
    return nc


def _meta_key(meta):
    return (
        tuple(int(v) for v in meta["grp_nb"]),
        tuple(len(v) for v in meta["slot_views"]),
        int(meta["NVIEWS"]),
    )


def _get_nc(meta):
    key = _meta_key(meta)
    if key not in _NC_CACHE:
        _NC_CACHE[key] = _build_nc(meta)
    return _NC_CACHE[key]


def kernel(x, edge_index, Wc, bc, W0, Wt):
    global _LAST_RESULTS
    from concourse.bass_utils import run_bass_kernel_spmd

    x = np.asarray(x)
    n = x.shape[0]
    in_maps, meta = _host_prep(x, edge_index, Wc, bc, W0, Wt)
    nc = _get_nc(meta)
    res = run_bass_kernel_spmd(nc, in_maps, list(range(N_CORES)))
    _LAST_RESULTS = res
    out_full = np.empty((N_PAD, D), np.float32)
    for k in range(N_CORES):
        out_full[meta["node_ids"][k]] = np.asarray(
            res.results[k]["out"]).astype(np.float32).T
    return out_full[:n].astype(np.float32)


# revision 4
# speedup vs baseline: 1.2921x; 1.0019x over previous
"""DGCN on 8 Trainium2 NeuronCores, v3: shared-boundary max-profile packing.

Differences vs v2:
  - Within a (group, half) gather region, slots are packed back-to-back at
    their max-over-cores edge counts (no per-slot ceil-to-128); only each
    region is block-aligned. Blocks spanning a slot boundary get one
    matmul+one-hot per covered slot (per-view dl columns).
  - Octets balanced by total edge count (primary) then lo count.
  - Flexible group sizes, ending with a tiny group so the post-gather
    pipeline tail is short.
"""

import numpy as np

N_NODES = 50000
D = 128
N_CORES = 8
TILE = 128
BLK = 128
HALF = 32768
MAX_IDX_CALL = 1024  # per dma_gather call (single_packet=False)
N_SWDGE_QUEUES = 4

N_TILES = 392
TPC = N_TILES // N_CORES          # 49
GROUP_SIZES = [12, 12, 12, 12, 1]  # slots per group; tiny last = short tail
N_PAD = N_TILES * TILE
ROWS_PC = TPC * TILE

_NC_CACHE = {}
_LAST_RESULTS = None


def _host_prep(x, edge_index, Wc, bc, W0, Wt):
    import ml_dtypes

    bf16 = ml_dtypes.bfloat16
    n, d = x.shape
    src = np.asarray(edge_index[0], dtype=np.int64)
    dst = np.asarray(edge_index[1], dtype=np.int64)

    deg = (np.bincount(dst, minlength=N_PAD) + 1).astype(np.float32)
    dinv = (1.0 / np.sqrt(deg)).astype(np.float32)

    src_a = src
    dst_a = dst

    tile_g = dst_a // TILE
    order0 = np.lexsort((src_a, tile_g))
    src_s = src_a[order0]
    dst_s = dst_a[order0]
    tile_s = tile_g[order0]

    counts = np.bincount(tile_s, minlength=N_TILES)
    starts = np.zeros(N_TILES + 1, np.int64)
    starts[1:] = np.cumsum(counts)
    lo_counts = np.empty(N_TILES, np.int64)
    for g in range(N_TILES):
        s0, c = starts[g], counts[g]
        lo_counts[g] = np.searchsorted(src_s[s0:s0 + c], HALF)
    hi_counts = counts - lo_counts

    # octet balancing: sort tiles by (lo, hi) DESCENDING and deal octet i
    # across the 8 cores as slot i; biggest slots first so the final (tiny)
    # group holds the smallest tiles. Then a pairwise same-core swap
    # hill-climb tightens sum(max lo + max hi) over octets.
    order_t = np.lexsort((hi_counts, lo_counts))[::-1]
    assign = order_t.reshape(TPC, N_CORES).copy()   # [slot, core] -> tile
    for _sweep in range(3):
        improved = 0
        alo = lo_counts[assign]
        ahi = hi_counts[assign]
        for i in range(TPC):
            for j in range(i + 1, TPC):
                base_cost = (alo[i].max() + ahi[i].max()
                             + alo[j].max() + ahi[j].max())
                for k in range(N_CORES):
                    alo[i, k], alo[j, k] = alo[j, k], alo[i, k]
                    ahi[i, k], ahi[j, k] = ahi[j, k], ahi[i, k]
                    new_cost = (alo[i].max() + ahi[i].max()
                                + alo[j].max() + ahi[j].max())
                    if new_cost < base_cost:
                        assign[i, k], assign[j, k] = assign[j, k], assign[i, k]
                        base_cost = new_cost
                        improved += 1
                    else:
                        alo[i, k], alo[j, k] = alo[j, k], alo[i, k]
                        ahi[i, k], ahi[j, k] = ahi[j, k], ahi[i, k]
        if improved == 0:
            break
    L_lo = lo_counts[assign].max(axis=1)            # [slot] max-profile lens
    L_hi = hi_counts[assign].max(axis=1)

    assert sum(GROUP_SIZES) == TPC
    grp_slots = []
    s0_ = 0
    for gs in GROUP_SIZES:
        grp_slots.append(list(range(s0_, s0_ + gs)))
        s0_ += gs

    # region/block/view layout
    NBASE = 0                    # running global block count
    callplan = []                # per group: (is_hi, local_b0, nb_blocks)
    grp_nb = []                  # blocks per group
    grp_off = []                 # global block offset per group
    slot_views_h = [([], []) for _ in range(TPC)]  # per half: (global_block, dl_col)
    # per-slot placement info for table building:
    place = {}                   # (slot, half) -> (region_pos, region_glb_b0)
    nview = 0
    for q, sl in enumerate(grp_slots):
        grp_off.append(NBASE)
        calls = []
        gb = 0                   # group-local block counter
        for h, L in ((0, L_lo), (1, L_hi)):
            pos = 0
            covered = []         # (slot, p0, p1) in region slot-positions
            for i in sl:
                li = int(L[i])
                place[(i, h)] = (pos, NBASE + gb)
                if li > 0:
                    covered.append((i, pos, pos + li))
                pos += li
            rblocks = -(-pos // BLK)
            # views: block b covers slot i iff ranges overlap
            for b in range(rblocks):
                lo_p, hi_p = b * BLK, (b + 1) * BLK
                for (i, p0, p1) in covered:
                    if p0 < hi_p and p1 > lo_p:
                        slot_views_h[i][h].append((NBASE + gb + b, nview))
                        nview += 1
            # calls (block-aligned, <=8 blocks each)
            b0 = gb
            nb = rblocks
            while nb > 0:
                take = min(nb, MAX_IDX_CALL // BLK)
                calls.append((h, b0, take))
                b0 += take
                nb -= take
            gb += rblocks
        grp_nb.append(gb)
        NBASE += gb
        callplan.append(calls)
    NB = NBASE
    NVIEWS = nview

    # tables
    idx_flat = np.zeros((N_CORES, NB * BLK), np.int32)
    dd_t = np.zeros((N_CORES, BLK, NB), np.float32)
    dl_t = np.full((N_CORES, BLK, NVIEWS), 1000.0, np.float32)
    for k in range(N_CORES):
        for i in range(TPC):
            g = assign[i, k]
            s0 = int(starts[g])
            clo, chi = int(lo_counts[g]), int(hi_counts[g])
            base = g * TILE
            for h, cnt, shift, pos0 in ((0, clo, 0, s0), (1, chi, HALF, s0 + clo)):
                if cnt == 0:
                    continue
                rpos, rgb0 = place[(i, h)]
                e0 = rgb0 * BLK + rpos
                idx_flat[k, e0:e0 + cnt] = src_s[pos0:pos0 + cnt] - shift
                # dd_t is [BLK, NB] with flat pos j -> [j%BLK, j//BLK]
                jj = np.arange(e0, e0 + cnt)
                dd_t[k][jj % BLK, jj // BLK] = dinv[dst_s[pos0:pos0 + cnt]]
        # dl per view
        for i in range(TPC):
            g = assign[i, k]
            s0 = int(starts[g])
            clo, chi = int(lo_counts[g]), int(hi_counts[g])
            base = g * TILE
            for (gb, vcol) in slot_views_h[i][0] + slot_views_h[i][1]:
                # which half does this view belong to?
                # find via placement: check lo then hi range
                done = False
                for h, cnt, pos0 in ((0, clo, s0), (1, chi, s0 + clo)):
                    rpos, rgb0 = place.get((i, h), (None, None))
                    if rpos is None:
                        continue
                    li = int((L_lo if h == 0 else L_hi)[i])
                    b_lo = rgb0 * BLK + rpos          # abs slot-pos of slot start
                    b_hi = b_lo + li
                    blk_lo, blk_hi = gb * BLK, (gb + 1) * BLK
                    if b_lo < blk_hi and b_hi > blk_lo:
                        # positions of this block within the slot's edge list
                        p_start = max(b_lo, blk_lo)
                        p_end = min(b_hi, blk_hi)
                        # rows within the block
                        r0, r1 = p_start - blk_lo, p_end - blk_lo
                        # edge offsets within slot's count
                        eo0 = p_start - b_lo
                        m = min(cnt - eo0, r1 - r0)
                        if m > 0:
                            dl_t[k][r0:r0 + m, vcol] = (
                                dst_s[pos0 + eo0:pos0 + eo0 + m] - base
                            ).astype(np.float32)
                        done = True
                if not done:
                    pass

    W16 = NB * (BLK // 16)
    ix16 = np.empty((N_CORES, BLK, W16), np.int16)
    for k in range(N_CORES):
        v = idx_flat[k].astype(np.int16).reshape(W16, 16).T
        ix16[k] = np.tile(v, (8, 1))

    Wc64 = np.asarray(Wc, np.float64)
    Wt64 = np.asarray(Wt, np.float64)
    W064 = np.asarray(W0, np.float64)
    bc64 = np.asarray(bc, np.float64)
    B1 = np.eye(d) + Wt64
    C = (Wc64 @ B1).astype(np.float32)
    B2 = (W064 - Wt64).astype(np.float32)
    bp = (bc64 @ B1).astype(np.float32)

    x_pad = np.zeros((N_PAD, d), np.float32)
    x_pad[:n] = np.asarray(x, np.float32)
    xs = (x_pad * dinv[:, None]).astype(bf16)
    xself = x_pad * (dinv * dinv)[:, None]

    iota = np.broadcast_to(np.arange(TILE, dtype=np.float32),
                           (BLK, TILE)).astype(bf16)

    node_ids = np.empty((N_CORES, ROWS_PC), np.int64)
    for k in range(N_CORES):
        for i in range(TPC):
            g = assign[i, k]
            node_ids[k, i * TILE:(i + 1) * TILE] = np.arange(
                g * TILE, (g + 1) * TILE)

    in_maps = []
    for k in range(N_CORES):
        m = {
            "x_lo": xs[:HALF],
            "x_hi": xs[HALF:],
            "xT": np.ascontiguousarray(x_pad[node_ids[k]].T.astype(bf16)),
            "xsT": np.ascontiguousarray(xself[node_ids[k]].T.astype(bf16)),
            "ix16": ix16[k],
            "dl": dl_t[k].astype(bf16),
            "dd": dd_t[k].astype(bf16),
            "cw": C.astype(bf16),
            "b2w": B2.astype(bf16),
            "bpc": bp.reshape(d, 1),
            "iota": np.ascontiguousarray(iota),
        }
        in_maps.append(m)

    meta = dict(
        grp_slots=grp_slots, grp_nb=grp_nb, grp_off=grp_off,
        callplan=callplan, slot_views=slot_views_h, NB=NB, NVIEWS=NVIEWS,
        W16=W16, hi_rows=N_PAD - HALF, node_ids=node_ids,
    )
    return in_maps, meta


def _build_nc(meta, ablate=()):
    import concourse.bacc as bacc
    import concourse.mybir as mybir
    import concourse.tile as tile
    from concourse import library_config

    f32 = mybir.dt.float32
    bf16 = mybir.dt.bfloat16
    i16 = mybir.dt.int16
    eq, mul = mybir.AluOpType.is_equal, mybir.AluOpType.mult
    ident = mybir.ActivationFunctionType.Identity

    grp_slots, grp_nb, grp_off = meta["grp_slots"], meta["grp_nb"], meta["grp_off"]
    callplan, slot_views = meta["callplan"], meta["slot_views"]
    NB, NVIEWS, W16 = meta["NB"], meta["NVIEWS"], meta["W16"]

    nc = bacc.Bacc(
        "TRN2",
        target_bir_lowering=False,
        debug=False,
        num_devices=N_CORES,
        num_swdge_queues=N_SWDGE_QUEUES,
    )
    x_lo = nc.declare_dram_parameter("x_lo", [HALF, D], bf16, isOutput=False)
    x_hi = nc.declare_dram_parameter("x_hi", [meta["hi_rows"], D], bf16,
                                     isOutput=False)
    xT = nc.declare_dram_parameter("xT", [D, ROWS_PC], bf16, isOutput=False)
    xsT = nc.declare_dram_parameter("xsT", [D, ROWS_PC], bf16, isOutput=False)
    ix16 = nc.declare_dram_parameter("ix16", [BLK, W16], i16, isOutput=False)
    dl = nc.declare_dram_parameter("dl", [BLK, NVIEWS], bf16, isOutput=False)
    dd = nc.declare_dram_parameter("dd", [BLK, NB], bf16, isOutput=False)
    cw = nc.declare_dram_parameter("cw", [D, D], bf16, isOutput=False)
    b2w = nc.declare_dram_parameter("b2w", [D, D], bf16, isOutput=False)
    bpc = nc.declare_dram_parameter("bpc", [D, 1], f32, isOutput=False)
    iota = nc.declare_dram_parameter("iota", [BLK, TILE], bf16, isOutput=False)
    out = nc.declare_dram_parameter("out", [D, ROWS_PC], bf16, isOutput=True)

    with tile.TileContext(nc) as tc:
        with (
            tc.tile_pool(name="const", bufs=1) as cpool,
            tc.tile_pool(name="tbl", bufs=1) as tpool,
            tc.tile_pool(name="gather", bufs=2) as gpool,
            tc.tile_pool(name="oh", bufs=4) as ohpool,
            tc.tile_pool(name="z", bufs=24) as zsbpool,
            tc.tile_pool(name="og", bufs=2) as ogpool,
            tc.tile_pool(name="zps", bufs=6, space="PSUM") as zpool,
            tc.tile_pool(name="ops", bufs=2, space="PSUM") as opool,
        ):
            nc.gpsimd.load_library(library_config.mlp)
            c_sb = cpool.tile([D, D], bf16)
            nc.sync.dma_start(out=c_sb[:], in_=cw[:])
            b2_sb = cpool.tile([D, D], bf16)
            nc.sync.dma_start(out=b2_sb[:], in_=b2w[:])
            bp_sb = cpool.tile([D, 1], f32)
            nc.sync.dma_start(out=bp_sb[:], in_=bpc[:])
            io_sb = cpool.tile([BLK, TILE], bf16)
            nc.sync.dma_start(out=io_sb[:], in_=iota[:])
            xT_sb = cpool.tile([D, ROWS_PC], bf16)
            ix_sb = tpool.tile([BLK, W16], i16)
            sl0 = min(16, int(grp_nb[0])) * 8
            nc.sync.dma_start(out=ix_sb[:, :sl0], in_=ix16[:, :sl0])
            dl_lb = tpool.tile([BLK, NVIEWS], bf16)
            nc.sync.dma_start(out=dl_lb[:], in_=dl[:])
            dd_lb = tpool.tile([BLK, NB], bf16)
            nc.sync.dma_start(out=dd_lb[:], in_=dd[:])
            dl_sb = tpool.tile([BLK, NVIEWS], f32)
            nc.vector.tensor_copy(out=dl_sb[:], in_=dl_lb[:])
            dd_sb = tpool.tile([BLK, NB], f32)
            nc.vector.tensor_copy(out=dd_sb[:], in_=dd_lb[:])
            if int(grp_nb[0]) * 8 > sl0:
                nc.sync.dma_start(out=ix_sb[:, sl0:int(grp_nb[0]) * 8],
                                  in_=ix16[:, sl0:int(grp_nb[0]) * 8])
            for q in range(1, len(grp_slots)):
                a, b = int(grp_off[q]) * 8, (int(grp_off[q]) + int(grp_nb[q])) * 8
                nc.sync.dma_start(out=ix_sb[:, a:b], in_=ix16[:, a:b])
            nc.sync.dma_start(out=xT_sb[:], in_=xT[:])
            xsT_sb = cpool.tile([D, ROWS_PC], bf16)
            nc.sync.dma_start(out=xsT_sb[:], in_=xsT[:])

            qrr = [0]
            for q, sl in enumerate(grp_slots):
                gb0 = int(grp_off[q])
                gnb = int(grp_nb[q])
                g_sb = gpool.tile([BLK, gnb * D], bf16, tag="g")
                if "gather" not in ablate:
                    for (is_hi, b0, nb) in callplan[q]:
                        tbl = x_hi if is_hi else x_lo
                        nidx = nb * BLK
                        nc.gpsimd.dma_gather(
                            out_ap=g_sb[:, b0 * D:(b0 + nb) * D].rearrange(
                                "p (n e) -> p n e", e=D),
                            in_ap=tbl[:],
                            idxs_ap=ix_sb[:, (gb0 + b0) * 8:(gb0 + b0 + nb) * 8],
                            num_idxs=nidx,
                            num_idxs_reg=nidx,
                            elem_size=D,
                            queue_num=qrr[0] % N_SWDGE_QUEUES,
                            single_packet=False,
                        )
                        qrr[0] += 1
                og_sb = ogpool.tile([D, len(sl) * TILE], bf16, tag="og")
                zparts = {}
                for h in (0, 1):
                    for i in sl:
                        views = slot_views[i][h]
                        if not views or "segmm" in ablate:
                            continue
                        nv = len(views)
                        z_ps = zpool.tile([D, TILE], f32)
                        for jj, (gb, vcol) in enumerate(views):
                            lb = gb - gb0
                            if "onehot" not in ablate:
                                oh = ohpool.tile([BLK, TILE], bf16, tag="oh")
                                nc.vector.tensor_scalar(
                                    out=oh[:], in0=io_sb[:],
                                    scalar1=dl_sb[:, vcol:vcol + 1],
                                    scalar2=dd_sb[:, gb:gb + 1],
                                    op0=eq, op1=mul,
                                )
                                rhs = oh[:]
                            else:
                                rhs = io_sb[:]
                            nc.tensor.matmul(
                                out=z_ps[:],
                                lhsT=g_sb[:, lb * D:(lb + 1) * D],
                                rhs=rhs,
                                start=(jj == 0), stop=(jj == nv - 1),
                            )
                        z_sb = zsbpool.tile([D, TILE], bf16, tag="z")
                        nc.scalar.copy(out=z_sb[:], in_=z_ps[:])
                        zparts.setdefault(i, []).append(z_sb)
                if "epilogue" not in ablate:
                    for s, i in enumerate(sl):
                        o_ps = opool.tile([D, TILE], f32)
                        parts = zparts.get(i, [])
                        for z_sb in parts:
                            nc.tensor.matmul(out=o_ps[:], lhsT=c_sb[:],
                                             rhs=z_sb[:],
                                             start=(z_sb is parts[0]),
                                             stop=False)
                        nc.tensor.matmul(out=o_ps[:], lhsT=c_sb[:],
                                         rhs=xsT_sb[:, i * TILE:(i + 1) * TILE],
                                         start=(not parts), stop=False)
                        nc.tensor.matmul(out=o_ps[:], lhsT=b2_sb[:],
                                         rhs=xT_sb[:, i * TILE:(i + 1) * TILE],
                                         start=False, stop=True)
                        nc.scalar.activation(
                            out=og_sb[:, s * TILE:(s + 1) * TILE], in_=o_ps[:],
                            func=ident, bias=bp_sb[:, 0:1])
                if "epilogue" not in ablate:
                    col0 = sl[0] * TILE
                    nc.sync.dma_start(
                        out=out[:, col0:col0 + len(sl) * TILE], in_=og_sb[:])
    nc.compile()
    return nc


def _meta_key(meta):
    return (
        tuple(int(v) for v in meta["grp_nb"]),
        tuple(len(v) for v in meta["slot_views"]),
        int(meta["NVIEWS"]),
    )


def _get_nc(meta):
    key = _meta_key(meta)
    if key not in _NC_CACHE:
        _NC_CACHE[key] = _build_nc(meta)
    return _NC_CACHE[key]


def kernel(x, edge_index, Wc, bc, W0, Wt):
    global _LAST_RESULTS
    from concourse.bass_utils import run_bass_kernel_spmd

    x = np.asarray(x)
    n = x.shape[0]
    in_maps, meta = _host_prep(x, edge_index, Wc, bc, W0, Wt)
    nc = _get_nc(meta)
    res = run_bass_kernel_spmd(nc, in_maps, list(range(N_CORES)))
    _LAST_RESULTS = res
    out_full = np.empty((N_PAD, D), np.float32)
    for k in range(N_CORES):
        out_full[meta["node_ids"][k]] = np.asarray(
            res.results[k]["out"]).astype(np.float32).T
    return out_full[:n].astype(np.float32)


# revision 5
# speedup vs baseline: 1.2940x; 1.0015x over previous
"""DGCN on 8 Trainium2 NeuronCores, v3: shared-boundary max-profile packing.

Differences vs v2:
  - Within a (group, half) gather region, slots are packed back-to-back at
    their max-over-cores edge counts (no per-slot ceil-to-128); only each
    region is block-aligned. Blocks spanning a slot boundary get one
    matmul+one-hot per covered slot (per-view dl columns).
  - Octets balanced by total edge count (primary) then lo count.
  - Flexible group sizes, ending with a tiny group so the post-gather
    pipeline tail is short.
"""

import numpy as np

N_NODES = 50000
D = 128
N_CORES = 8
TILE = 128
BLK = 128
HALF = 32768
MAX_IDX_CALL = 1024  # per dma_gather call
N_SWDGE_QUEUES = 4
OFFLOAD_K = 0        # one-hot gpsimd offload disabled (head-of-line blocks gathers)

N_TILES = 392
TPC = N_TILES // N_CORES          # 49
GROUP_SIZES = [12, 12, 12, 12, 1]  # slots per group; tiny last = short tail
N_PAD = N_TILES * TILE
ROWS_PC = TPC * TILE

_NC_CACHE = {}
_LAST_RESULTS = None


def _host_prep(x, edge_index, Wc, bc, W0, Wt):
    import ml_dtypes

    bf16 = ml_dtypes.bfloat16
    n, d = x.shape
    src = np.asarray(edge_index[0], dtype=np.int64)
    dst = np.asarray(edge_index[1], dtype=np.int64)

    deg = (np.bincount(dst, minlength=N_PAD) + 1).astype(np.float32)
    dinv = (1.0 / np.sqrt(deg)).astype(np.float32)

    src_a = src
    dst_a = dst

    tile_g = dst_a // TILE
    order0 = np.lexsort((src_a, tile_g))
    src_s = src_a[order0]
    dst_s = dst_a[order0]
    tile_s = tile_g[order0]

    counts = np.bincount(tile_s, minlength=N_TILES)
    starts = np.zeros(N_TILES + 1, np.int64)
    starts[1:] = np.cumsum(counts)
    lo_counts = np.empty(N_TILES, np.int64)
    for g in range(N_TILES):
        s0, c = starts[g], counts[g]
        lo_counts[g] = np.searchsorted(src_s[s0:s0 + c], HALF)
    hi_counts = counts - lo_counts

    # octet balancing: sort tiles by (lo, hi) DESCENDING and deal octet i
    # across the 8 cores as slot i; biggest slots first so the final (tiny)
    # group holds the smallest tiles. Then a pairwise same-core swap
    # hill-climb tightens sum(max lo + max hi) over octets.
    order_t = np.lexsort((hi_counts, lo_counts))[::-1]
    assign = order_t.reshape(TPC, N_CORES).copy()   # [slot, core] -> tile
    for _sweep in range(3):
        improved = 0
        alo = lo_counts[assign]
        ahi = hi_counts[assign]
        for i in range(TPC):
            for j in range(i + 1, TPC):
                base_cost = (alo[i].max() + ahi[i].max()
                             + alo[j].max() + ahi[j].max())
                for k in range(N_CORES):
                    alo[i, k], alo[j, k] = alo[j, k], alo[i, k]
                    ahi[i, k], ahi[j, k] = ahi[j, k], ahi[i, k]
                    new_cost = (alo[i].max() + ahi[i].max()
                                + alo[j].max() + ahi[j].max())
                    if new_cost < base_cost:
                        assign[i, k], assign[j, k] = assign[j, k], assign[i, k]
                        base_cost = new_cost
                        improved += 1
                    else:
                        alo[i, k], alo[j, k] = alo[j, k], alo[i, k]
                        ahi[i, k], ahi[j, k] = ahi[j, k], ahi[i, k]
        if improved == 0:
            break
    L_lo = lo_counts[assign].max(axis=1)            # [slot] max-profile lens
    L_hi = hi_counts[assign].max(axis=1)

    assert sum(GROUP_SIZES) == TPC
    grp_slots = []
    s0_ = 0
    for gs in GROUP_SIZES:
        grp_slots.append(list(range(s0_, s0_ + gs)))
        s0_ += gs

    # region/block/view layout
    NBASE = 0                    # running global block count
    callplan = []                # per group: (is_hi, local_b0, nb_blocks)
    grp_nb = []                  # blocks per group
    grp_off = []                 # global block offset per group
    slot_views_h = [([], []) for _ in range(TPC)]  # per half: (global_block, dl_col)
    # per-slot placement info for table building:
    place = {}                   # (slot, half) -> (region_pos, region_glb_b0)
    nview = 0
    for q, sl in enumerate(grp_slots):
        grp_off.append(NBASE)
        calls = []
        gb = 0                   # group-local block counter
        for h, L in ((0, L_lo), (1, L_hi)):
            pos = 0
            covered = []         # (slot, p0, p1) in region slot-positions
            for i in sl:
                li = int(L[i])
                place[(i, h)] = (pos, NBASE + gb)
                if li > 0:
                    covered.append((i, pos, pos + li))
                pos += li
            rblocks = -(-pos // BLK)
            # views: block b covers slot i iff ranges overlap
            for b in range(rblocks):
                lo_p, hi_p = b * BLK, (b + 1) * BLK
                for (i, p0, p1) in covered:
                    if p0 < hi_p and p1 > lo_p:
                        slot_views_h[i][h].append((NBASE + gb + b, nview))
                        nview += 1
            # calls (block-aligned, <=8 blocks each)
            b0 = gb
            nb = rblocks
            while nb > 0:
                take = min(nb, MAX_IDX_CALL // BLK)
                calls.append((h, b0, take))
                b0 += take
                nb -= take
            gb += rblocks
        grp_nb.append(gb)
        NBASE += gb
        callplan.append(calls)
    NB = NBASE
    NVIEWS = nview

    # tables
    idx_flat = np.zeros((N_CORES, NB * BLK), np.int32)
    dd_t = np.zeros((N_CORES, BLK, NB), np.float32)
    dl_t = np.full((N_CORES, BLK, NVIEWS), 1000.0, np.float32)
    for k in range(N_CORES):
        for i in range(TPC):
            g = assign[i, k]
            s0 = int(starts[g])
            clo, chi = int(lo_counts[g]), int(hi_counts[g])
            base = g * TILE
            for h, cnt, shift, pos0 in ((0, clo, 0, s0), (1, chi, HALF, s0 + clo)):
                if cnt == 0:
                    continue
                rpos, rgb0 = place[(i, h)]
                e0 = rgb0 * BLK + rpos
                idx_flat[k, e0:e0 + cnt] = src_s[pos0:pos0 + cnt] - shift
                # dd_t is [BLK, NB] with flat pos j -> [j%BLK, j//BLK]
                jj = np.arange(e0, e0 + cnt)
                dd_t[k][jj % BLK, jj // BLK] = dinv[dst_s[pos0:pos0 + cnt]]
        # dl per view
        for i in range(TPC):
            g = assign[i, k]
            s0 = int(starts[g])
            clo, chi = int(lo_counts[g]), int(hi_counts[g])
            base = g * TILE
            for (gb, vcol) in slot_views_h[i][0] + slot_views_h[i][1]:
                # which half does this view belong to?
                # find via placement: check lo then hi range
                done = False
                for h, cnt, pos0 in ((0, clo, s0), (1, chi, s0 + clo)):
                    rpos, rgb0 = place.get((i, h), (None, None))
                    if rpos is None:
                        continue
                    li = int((L_lo if h == 0 else L_hi)[i])
                    b_lo = rgb0 * BLK + rpos          # abs slot-pos of slot start
                    b_hi = b_lo + li
                    blk_lo, blk_hi = gb * BLK, (gb + 1) * BLK
                    if b_lo < blk_hi and b_hi > blk_lo:
                        # positions of this block within the slot's edge list
                        p_start = max(b_lo, blk_lo)
                        p_end = min(b_hi, blk_hi)
                        # rows within the block
                        r0, r1 = p_start - blk_lo, p_end - blk_lo
                        # edge offsets within slot's count
                        eo0 = p_start - b_lo
                        m = min(cnt - eo0, r1 - r0)
                        if m > 0:
                            dl_t[k][r0:r0 + m, vcol] = (
                                dst_s[pos0 + eo0:pos0 + eo0 + m] - base
                            ).astype(np.float32)
                        done = True
                if not done:
                    pass

    W16 = NB * (BLK // 16)
    ix16 = np.empty((N_CORES, BLK, W16), np.int16)
    for k in range(N_CORES):
        v = idx_flat[k].astype(np.int16).reshape(W16, 16).T
        ix16[k] = np.tile(v, (8, 1))

    Wc64 = np.asarray(Wc, np.float64)
    Wt64 = np.asarray(Wt, np.float64)
    W064 = np.asarray(W0, np.float64)
    bc64 = np.asarray(bc, np.float64)
    B1 = np.eye(d) + Wt64
    C = (Wc64 @ B1).astype(np.float32)
    B2 = (W064 - Wt64).astype(np.float32)
    bp = (bc64 @ B1).astype(np.float32)

    x_pad = np.zeros((N_PAD, d), np.float32)
    x_pad[:n] = np.asarray(x, np.float32)
    xs = (x_pad * dinv[:, None]).astype(bf16)
    xself = x_pad * (dinv * dinv)[:, None]

    iota = np.broadcast_to(np.arange(TILE, dtype=np.float32),
                           (BLK, TILE)).astype(bf16)

    node_ids = np.empty((N_CORES, ROWS_PC), np.int64)
    for k in range(N_CORES):
        for i in range(TPC):
            g = assign[i, k]
            node_ids[k, i * TILE:(i + 1) * TILE] = np.arange(
                g * TILE, (g + 1) * TILE)

    in_maps = []
    for k in range(N_CORES):
        m = {
            "x_lo": xs[:HALF],
            "x_hi": xs[HALF:],
            "xT": np.ascontiguousarray(x_pad[node_ids[k]].T.astype(bf16)),
            "xsT": np.ascontiguousarray(xself[node_ids[k]].T.astype(bf16)),
            "ix16": ix16[k],
            "dl": dl_t[k].astype(bf16),
            "dd": dd_t[k].astype(bf16),
            "cw": C.astype(bf16),
            "b2w": B2.astype(bf16),
            "bpc": bp.reshape(d, 1),
            "iota": np.ascontiguousarray(iota),
        }
        in_maps.append(m)

    meta = dict(
        grp_slots=grp_slots, grp_nb=grp_nb, grp_off=grp_off,
        callplan=callplan, slot_views=slot_views_h, NB=NB, NVIEWS=NVIEWS,
        W16=W16, hi_rows=N_PAD - HALF, node_ids=node_ids,
    )
    return in_maps, meta


def _build_nc(meta, ablate=()):
    import concourse.bacc as bacc
    import concourse.mybir as mybir
    import concourse.tile as tile
    from concourse import library_config

    f32 = mybir.dt.float32
    bf16 = mybir.dt.bfloat16
    i16 = mybir.dt.int16
    eq, mul = mybir.AluOpType.is_equal, mybir.AluOpType.mult
    ident = mybir.ActivationFunctionType.Identity

    grp_slots, grp_nb, grp_off = meta["grp_slots"], meta["grp_nb"], meta["grp_off"]
    callplan, slot_views = meta["callplan"], meta["slot_views"]
    NB, NVIEWS, W16 = meta["NB"], meta["NVIEWS"], meta["W16"]

    nc = bacc.Bacc(
        "TRN2",
        target_bir_lowering=False,
        debug=False,
        num_devices=N_CORES,
        num_swdge_queues=N_SWDGE_QUEUES,
    )
    x_lo = nc.declare_dram_parameter("x_lo", [HALF, D], bf16, isOutput=False)
    x_hi = nc.declare_dram_parameter("x_hi", [meta["hi_rows"], D], bf16,
                                     isOutput=False)
    xT = nc.declare_dram_parameter("xT", [D, ROWS_PC], bf16, isOutput=False)
    xsT = nc.declare_dram_parameter("xsT", [D, ROWS_PC], bf16, isOutput=False)
    ix16 = nc.declare_dram_parameter("ix16", [BLK, W16], i16, isOutput=False)
    dl = nc.declare_dram_parameter("dl", [BLK, NVIEWS], bf16, isOutput=False)
    dd = nc.declare_dram_parameter("dd", [BLK, NB], bf16, isOutput=False)
    cw = nc.declare_dram_parameter("cw", [D, D], bf16, isOutput=False)
    b2w = nc.declare_dram_parameter("b2w", [D, D], bf16, isOutput=False)
    bpc = nc.declare_dram_parameter("bpc", [D, 1], f32, isOutput=False)
    iota = nc.declare_dram_parameter("iota", [BLK, TILE], bf16, isOutput=False)
    out = nc.declare_dram_parameter("out", [D, ROWS_PC], bf16, isOutput=True)

    with tile.TileContext(nc) as tc:
        with (
            tc.tile_pool(name="const", bufs=1) as cpool,
            tc.tile_pool(name="tbl", bufs=1) as tpool,
            tc.tile_pool(name="gather", bufs=2) as gpool,
            tc.tile_pool(name="oh", bufs=16) as ohpool,
            tc.tile_pool(name="z", bufs=24) as zsbpool,
            tc.tile_pool(name="og", bufs=2) as ogpool,
            tc.tile_pool(name="zps", bufs=6, space="PSUM") as zpool,
            tc.tile_pool(name="ops", bufs=2, space="PSUM") as opool,
        ):
            nc.gpsimd.load_library(library_config.mlp)
            c_sb = cpool.tile([D, D], bf16)
            nc.sync.dma_start(out=c_sb[:], in_=cw[:])
            b2_sb = cpool.tile([D, D], bf16)
            nc.sync.dma_start(out=b2_sb[:], in_=b2w[:])
            bp_sb = cpool.tile([D, 1], f32)
            nc.sync.dma_start(out=bp_sb[:], in_=bpc[:])
            io_sb = cpool.tile([BLK, TILE], bf16)
            nc.sync.dma_start(out=io_sb[:], in_=iota[:])
            xT_sb = cpool.tile([D, ROWS_PC], bf16)
            ix_sb = tpool.tile([BLK, W16], i16)
            sl0 = min(16, int(grp_nb[0])) * 8
            nc.sync.dma_start(out=ix_sb[:, :sl0], in_=ix16[:, :sl0])
            dl_lb = tpool.tile([BLK, NVIEWS], bf16)
            nc.sync.dma_start(out=dl_lb[:], in_=dl[:])
            dd_lb = tpool.tile([BLK, NB], bf16)
            nc.sync.dma_start(out=dd_lb[:], in_=dd[:])
            dl_sb = tpool.tile([BLK, NVIEWS], f32)
            nc.vector.tensor_copy(out=dl_sb[:], in_=dl_lb[:])
            dd_sb = tpool.tile([BLK, NB], f32)
            nc.vector.tensor_copy(out=dd_sb[:], in_=dd_lb[:])
            if int(grp_nb[0]) * 8 > sl0:
                nc.sync.dma_start(out=ix_sb[:, sl0:int(grp_nb[0]) * 8],
                                  in_=ix16[:, sl0:int(grp_nb[0]) * 8])
            for q in range(1, len(grp_slots)):
                a, b = int(grp_off[q]) * 8, (int(grp_off[q]) + int(grp_nb[q])) * 8
                nc.sync.dma_start(out=ix_sb[:, a:b], in_=ix16[:, a:b])
            nc.sync.dma_start(out=xT_sb[:], in_=xT[:])
            xsT_sb = cpool.tile([D, ROWS_PC], bf16)
            nc.sync.dma_start(out=xsT_sb[:], in_=xsT[:])

            qrr = [0]
            vcnt = [0]
            for q, sl in enumerate(grp_slots):
                gb0 = int(grp_off[q])
                gnb = int(grp_nb[q])
                g_sb = gpool.tile([BLK, gnb * D], bf16, tag="g")
                if "gather" not in ablate:
                    for (is_hi, b0, nb) in callplan[q]:
                        tbl = x_hi if is_hi else x_lo
                        nidx = nb * BLK
                        nc.gpsimd.dma_gather(
                            out_ap=g_sb[:, b0 * D:(b0 + nb) * D].rearrange(
                                "p (n e) -> p n e", e=D),
                            in_ap=tbl[:],
                            idxs_ap=ix_sb[:, (gb0 + b0) * 8:(gb0 + b0 + nb) * 8],
                            num_idxs=nidx,
                            num_idxs_reg=nidx,
                            elem_size=D,
                            queue_num=qrr[0] % N_SWDGE_QUEUES,
                            single_packet=False,
                        )
                        qrr[0] += 1
                og_sb = ogpool.tile([D, len(sl) * TILE], bf16, tag="og")
                zparts = {}
                for h in (0, 1):
                    for i in sl:
                        views = slot_views[i][h]
                        if not views or "segmm" in ablate:
                            continue
                        nv = len(views)
                        z_ps = zpool.tile([D, TILE], f32)
                        for jj, (gb, vcol) in enumerate(views):
                            lb = gb - gb0
                            if "onehot" not in ablate:
                                oh = ohpool.tile([BLK, TILE], bf16, tag="oh")
                                vcnt[0] += 1
                                eng = (nc.gpsimd if OFFLOAD_K and
                                       vcnt[0] % OFFLOAD_K == 0 else nc.vector)
                                eng.tensor_scalar(
                                    out=oh[:], in0=io_sb[:],
                                    scalar1=dl_sb[:, vcol:vcol + 1],
                                    scalar2=dd_sb[:, gb:gb + 1],
                                    op0=eq, op1=mul,
                                )
                                rhs = oh[:]
                            else:
                                rhs = io_sb[:]
                            nc.tensor.matmul(
                                out=z_ps[:],
                                lhsT=g_sb[:, lb * D:(lb + 1) * D],
                                rhs=rhs,
                                start=(jj == 0), stop=(jj == nv - 1),
                            )
                        z_sb = zsbpool.tile([D, TILE], bf16, tag="z")
                        nc.scalar.copy(out=z_sb[:], in_=z_ps[:])
                        zparts.setdefault(i, []).append(z_sb)
                if "epilogue" not in ablate:
                    for s, i in enumerate(sl):
                        o_ps = opool.tile([D, TILE], f32)
                        parts = zparts.get(i, [])
                        for z_sb in parts:
                            nc.tensor.matmul(out=o_ps[:], lhsT=c_sb[:],
                                             rhs=z_sb[:],
                                             start=(z_sb is parts[0]),
                                             stop=False)
                        nc.tensor.matmul(out=o_ps[:], lhsT=c_sb[:],
                                         rhs=xsT_sb[:, i * TILE:(i + 1) * TILE],
                                         start=(not parts), stop=False)
                        nc.tensor.matmul(out=o_ps[:], lhsT=b2_sb[:],
                                         rhs=xT_sb[:, i * TILE:(i + 1) * TILE],
                                         start=False, stop=True)
                        nc.scalar.activation(
                            out=og_sb[:, s * TILE:(s + 1) * TILE], in_=o_ps[:],
                            func=ident, bias=bp_sb[:, 0:1])
                if "epilogue" not in ablate:
                    col0 = sl[0] * TILE
                    nc.sync.dma_start(
                        out=out[:, col0:col0 + len(sl) * TILE], in_=og_sb[:])
    nc.compile()
    return nc


def _meta_key(meta):
    return (
        tuple(int(v) for v in meta["grp_nb"]),
        tuple(len(v) for v in meta["slot_views"]),
        int(meta["NVIEWS"]),
    )


def _get_nc(meta):
    key = _meta_key(meta)
    if key not in _NC_CACHE:
        _NC_CACHE[key] = _build_nc(meta)
    return _NC_CACHE[key]


def kernel(x, edge_index, Wc, bc, W0, Wt):
    global _LAST_RESULTS
    from concourse.bass_utils import run_bass_kernel_spmd

    x = np.asarray(x)
    n = x.shape[0]
    in_maps, meta = _host_prep(x, edge_index, Wc, bc, W0, Wt)
    nc = _get_nc(meta)
    res = run_bass_kernel_spmd(nc, in_maps, list(range(N_CORES)))
    _LAST_RESULTS = res
    out_full = np.empty((N_PAD, D), np.float32)
    for k in range(N_CORES):
        out_full[meta["node_ids"][k]] = np.asarray(
            res.results[k]["out"]).astype(np.float32).T
    return out_full[:n].astype(np.float32)


# revision 6
# speedup vs baseline: 1.3040x; 1.0077x over previous
"""DGCN on 8 Trainium2 NeuronCores, v3: shared-boundary max-profile packing.

Differences vs v2:
  - Within a (group, half) gather region, slots are packed back-to-back at
    their max-over-cores edge counts (no per-slot ceil-to-128); only each
    region is block-aligned. Blocks spanning a slot boundary get one
    matmul+one-hot per covered slot (per-view dl columns).
  - Octets balanced by total edge count (primary) then lo count.
  - Flexible group sizes, ending with a tiny group so the post-gather
    pipeline tail is short.
"""

import numpy as np

N_NODES = 50000
D = 128
N_CORES = 8
TILE = 128
BLK = 128
HALF = 32768
MAX_IDX_CALL = 1024  # per dma_gather call
N_SWDGE_QUEUES = 4
OFFLOAD_K = 0        # one-hot gpsimd offload disabled (head-of-line blocks gathers)

N_TILES = 392
TPC = N_TILES // N_CORES          # 49
GROUP_SIZES = [12, 12, 12, 12, 1]  # slots per group; tiny last = short tail
N_PAD = N_TILES * TILE
ROWS_PC = TPC * TILE

_NC_CACHE = {}
_LAST_RESULTS = None


def _host_prep(x, edge_index, Wc, bc, W0, Wt):
    import ml_dtypes

    bf16 = ml_dtypes.bfloat16
    n, d = x.shape
    src = np.asarray(edge_index[0], dtype=np.int64)
    dst = np.asarray(edge_index[1], dtype=np.int64)

    deg = (np.bincount(dst, minlength=N_PAD) + 1).astype(np.float32)
    dinv = (1.0 / np.sqrt(deg)).astype(np.float32)

    src_a = src
    dst_a = dst

    tile_g = dst_a // TILE
    ishi = (src_a >= HALF).astype(np.int64)
    order0 = np.lexsort((dst_a, ishi, tile_g))
    src_s = src_a[order0]
    dst_s = dst_a[order0]
    tile_s = tile_g[order0]

    counts = np.bincount(tile_s, minlength=N_TILES)
    starts = np.zeros(N_TILES + 1, np.int64)
    starts[1:] = np.cumsum(counts)
    lo_counts = np.bincount(tile_g[ishi == 0], minlength=N_TILES)
    hi_counts = counts - lo_counts

    # octet balancing: sort tiles by (lo, hi) DESCENDING and deal octet i
    # across the 8 cores as slot i; biggest slots first so the final (tiny)
    # group holds the smallest tiles. Then a pairwise same-core swap
    # hill-climb tightens sum(max lo + max hi) over octets.
    order_t = np.lexsort((hi_counts, lo_counts))[::-1]
    assign = order_t.reshape(TPC, N_CORES).copy()   # [slot, core] -> tile
    for _sweep in range(3):
        improved = 0
        alo = lo_counts[assign]
        ahi = hi_counts[assign]
        for i in range(TPC):
            for j in range(i + 1, TPC):
                base_cost = (alo[i].max() + ahi[i].max()
                             + alo[j].max() + ahi[j].max())
                for k in range(N_CORES):
                    alo[i, k], alo[j, k] = alo[j, k], alo[i, k]
                    ahi[i, k], ahi[j, k] = ahi[j, k], ahi[i, k]
                    new_cost = (alo[i].max() + ahi[i].max()
                                + alo[j].max() + ahi[j].max())
                    if new_cost < base_cost:
                        assign[i, k], assign[j, k] = assign[j, k], assign[i, k]
                        base_cost = new_cost
                        improved += 1
                    else:
                        alo[i, k], alo[j, k] = alo[j, k], alo[i, k]
                        ahi[i, k], ahi[j, k] = ahi[j, k], ahi[i, k]
        if improved == 0:
            break
    L_lo = lo_counts[assign].max(axis=1)            # [slot] max-profile lens
    L_hi = hi_counts[assign].max(axis=1)

    assert sum(GROUP_SIZES) == TPC
    grp_slots = []
    s0_ = 0
    for gs in GROUP_SIZES:
        grp_slots.append(list(range(s0_, s0_ + gs)))
        s0_ += gs

    # region/block/view layout
    NBASE = 0                    # running global block count
    callplan = []                # per group: (is_hi, local_b0, nb_blocks)
    grp_nb = []                  # blocks per group
    grp_off = []                 # global block offset per group
    slot_views_h = [([], []) for _ in range(TPC)]  # per half: (global_block, dl_col)
    # per-slot placement info for table building:
    place = {}                   # (slot, half) -> (region_pos, region_glb_b0)
    nview = 0
    for q, sl in enumerate(grp_slots):
        grp_off.append(NBASE)
        calls = []
        gb = 0                   # group-local block counter
        for h, L in ((0, L_lo), (1, L_hi)):
            pos = 0
            covered = []         # (slot, p0, p1) in region slot-positions
            for i in sl:
                li = int(L[i])
                place[(i, h)] = (pos, NBASE + gb)
                if li > 0:
                    covered.append((i, pos, pos + li))
                pos += li
            rblocks = -(-pos // BLK)
            # views: block b covers slot i iff ranges overlap
            for b in range(rblocks):
                lo_p, hi_p = b * BLK, (b + 1) * BLK
                for (i, p0, p1) in covered:
                    if p0 < hi_p and p1 > lo_p:
                        slot_views_h[i][h].append((NBASE + gb + b, nview))
                        nview += 1
            # calls (block-aligned, <=8 blocks each)
            b0 = gb
            nb = rblocks
            while nb > 0:
                take = min(nb, MAX_IDX_CALL // BLK)
                calls.append((h, b0, take))
                b0 += take
                nb -= take
            gb += rblocks
        grp_nb.append(gb)
        NBASE += gb
        callplan.append(calls)
    NB = NBASE
    NVIEWS = nview

    # tables
    idx_flat = np.zeros((N_CORES, NB * BLK), np.int32)
    dd_t = np.zeros((N_CORES, BLK, NB), np.float32)
    dl_t = np.full((N_CORES, BLK, NVIEWS), 1000.0, np.float32)
    for k in range(N_CORES):
        for i in range(TPC):
            g = assign[i, k]
            s0 = int(starts[g])
            clo, chi = int(lo_counts[g]), int(hi_counts[g])
            base = g * TILE
            for h, cnt, shift, pos0 in ((0, clo, 0, s0), (1, chi, HALF, s0 + clo)):
                if cnt == 0:
                    continue
                rpos, rgb0 = place[(i, h)]
                e0 = rgb0 * BLK + rpos
                idx_flat[k, e0:e0 + cnt] = src_s[pos0:pos0 + cnt] - shift
                # dd_t is [BLK, NB] with flat pos j -> [j%BLK, j//BLK]
                jj = np.arange(e0, e0 + cnt)
                dd_t[k][jj % BLK, jj // BLK] = dinv[dst_s[pos0:pos0 + cnt]]
        # dl per view (pass B below; geometry shared with pass A)

    def _view_rows(k):
        for i in range(TPC):
            g = assign[i, k]
            s0 = int(starts[g])
            clo, chi = int(lo_counts[g]), int(hi_counts[g])
            base = g * TILE
            for h in (0, 1):
                cnt = clo if h == 0 else chi
                pos0 = s0 if h == 0 else s0 + clo
                rpos, rgb0 = place.get((i, h), (None, None))
                if rpos is None:
                    continue
                li = int((L_lo if h == 0 else L_hi)[i])
                b_lo = rgb0 * BLK + rpos
                b_hi = b_lo + li
                for jj, (gb, vcol) in enumerate(slot_views_h[i][h]):
                    blk_lo, blk_hi = gb * BLK, (gb + 1) * BLK
                    if not (b_lo < blk_hi and b_hi > blk_lo):
                        continue
                    p_start = max(b_lo, blk_lo)
                    p_end = min(b_hi, blk_hi)
                    r0 = p_start - blk_lo
                    eo0 = p_start - b_lo
                    m = min(cnt - eo0, p_end - p_start)
                    yield (vcol, jj, r0, m, pos0 + eo0, base)

    d0_v = np.full(NVIEWS, TILE, np.int64)
    d1_v = np.zeros(NVIEWS, np.int64)
    first_v = np.zeros(NVIEWS, bool)
    for k in range(N_CORES):
        for (vcol, jj, r0, m, p0, base) in _view_rows(k):
            if jj == 0:
                first_v[vcol] = True
            if m > 0:
                dv = dst_s[p0:p0 + m] - base
                d0_v[vcol] = min(d0_v[vcol], int(dv.min()))
                d1_v[vcol] = max(d1_v[vcol], int(dv.max()) + 1)
    d0_v[first_v] = 0
    d1_v[first_v] = TILE
    bad = d1_v <= d0_v
    d0_v[bad] = 0
    d1_v[bad] = 1

    W16 = NB * (BLK // 16)
    for k in range(N_CORES):
        for (vcol, jj, r0, m, p0, base) in _view_rows(k):
            if m > 0:
                dl_t[k][r0:r0 + m, vcol] = (
                    dst_s[p0:p0 + m] - base - d0_v[vcol]).astype(np.float32)

    ix16 = np.empty((N_CORES, BLK, W16), np.int16)
    for k in range(N_CORES):
        v = idx_flat[k].astype(np.int16).reshape(W16, 16).T
        ix16[k] = np.tile(v, (8, 1))

    Wc64 = np.asarray(Wc, np.float64)
    Wt64 = np.asarray(Wt, np.float64)
    W064 = np.asarray(W0, np.float64)
    bc64 = np.asarray(bc, np.float64)
    B1 = np.eye(d) + Wt64
    C = (Wc64 @ B1).astype(np.float32)
    B2 = (W064 - Wt64).astype(np.float32)
    bp = (bc64 @ B1).astype(np.float32)

    x_pad = np.zeros((N_PAD, d), np.float32)
    x_pad[:n] = np.asarray(x, np.float32)
    xs = (x_pad * dinv[:, None]).astype(bf16)
    xself = x_pad * (dinv * dinv)[:, None]

    iota = np.broadcast_to(np.arange(TILE, dtype=np.float32),
                           (BLK, TILE)).astype(bf16)

    node_ids = np.empty((N_CORES, ROWS_PC), np.int64)
    for k in range(N_CORES):
        for i in range(TPC):
            g = assign[i, k]
            node_ids[k, i * TILE:(i + 1) * TILE] = np.arange(
                g * TILE, (g + 1) * TILE)

    in_maps = []
    for k in range(N_CORES):
        m = {
            "x_lo": xs[:HALF],
            "x_hi": xs[HALF:],
            "xT": np.ascontiguousarray(x_pad[node_ids[k]].T.astype(bf16)),
            "xsT": np.ascontiguousarray(xself[node_ids[k]].T.astype(bf16)),
            "ix16": ix16[k],
            "dl": dl_t[k].astype(bf16),
            "dd": dd_t[k].astype(bf16),
            "cw": C.astype(bf16),
            "b2w": B2.astype(bf16),
            "bpc": bp.reshape(d, 1),
            "iota": np.ascontiguousarray(iota),
        }
        in_maps.append(m)

    meta = dict(
        grp_slots=grp_slots, grp_nb=grp_nb, grp_off=grp_off,
        callplan=callplan, slot_views=slot_views_h, NB=NB, NVIEWS=NVIEWS,
        d0_v=d0_v, d1_v=d1_v,
        W16=W16, hi_rows=N_PAD - HALF, node_ids=node_ids,
    )
    return in_maps, meta


def _build_nc(meta, ablate=()):
    import concourse.bacc as bacc
    import concourse.mybir as mybir
    import concourse.tile as tile
    from concourse import library_config

    f32 = mybir.dt.float32
    bf16 = mybir.dt.bfloat16
    i16 = mybir.dt.int16
    eq, mul = mybir.AluOpType.is_equal, mybir.AluOpType.mult
    ident = mybir.ActivationFunctionType.Identity

    grp_slots, grp_nb, grp_off = meta["grp_slots"], meta["grp_nb"], meta["grp_off"]
    callplan, slot_views = meta["callplan"], meta["slot_views"]
    NB, NVIEWS, W16 = meta["NB"], meta["NVIEWS"], meta["W16"]
    d0_v, d1_v = meta["d0_v"], meta["d1_v"]

    nc = bacc.Bacc(
        "TRN2",
        target_bir_lowering=False,
        debug=False,
        num_devices=N_CORES,
        num_swdge_queues=N_SWDGE_QUEUES,
    )
    x_lo = nc.declare_dram_parameter("x_lo", [HALF, D], bf16, isOutput=False)
    x_hi = nc.declare_dram_parameter("x_hi", [meta["hi_rows"], D], bf16,
                                     isOutput=False)
    xT = nc.declare_dram_parameter("xT", [D, ROWS_PC], bf16, isOutput=False)
    xsT = nc.declare_dram_parameter("xsT", [D, ROWS_PC], bf16, isOutput=False)
    ix16 = nc.declare_dram_parameter("ix16", [BLK, W16], i16, isOutput=False)
    dl = nc.declare_dram_parameter("dl", [BLK, NVIEWS], bf16, isOutput=False)
    dd = nc.declare_dram_parameter("dd", [BLK, NB], bf16, isOutput=False)
    cw = nc.declare_dram_parameter("cw", [D, D], bf16, isOutput=False)
    b2w = nc.declare_dram_parameter("b2w", [D, D], bf16, isOutput=False)
    bpc = nc.declare_dram_parameter("bpc", [D, 1], f32, isOutput=False)
    iota = nc.declare_dram_parameter("iota", [BLK, TILE], bf16, isOutput=False)
    out = nc.declare_dram_parameter("out", [D, ROWS_PC], bf16, isOutput=True)

    with tile.TileContext(nc) as tc:
        with (
            tc.tile_pool(name="const", bufs=1) as cpool,
            tc.tile_pool(name="tbl", bufs=1) as tpool,
            tc.tile_pool(name="gather", bufs=2) as gpool,
            tc.tile_pool(name="oh", bufs=16) as ohpool,
            tc.tile_pool(name="z", bufs=24) as zsbpool,
            tc.tile_pool(name="og", bufs=2) as ogpool,
            tc.tile_pool(name="zps", bufs=6, space="PSUM") as zpool,
            tc.tile_pool(name="ops", bufs=2, space="PSUM") as opool,
        ):
            nc.gpsimd.load_library(library_config.mlp)
            c_sb = cpool.tile([D, D], bf16)
            nc.sync.dma_start(out=c_sb[:], in_=cw[:])
            b2_sb = cpool.tile([D, D], bf16)
            nc.sync.dma_start(out=b2_sb[:], in_=b2w[:])
            bp_sb = cpool.tile([D, 1], f32)
            nc.sync.dma_start(out=bp_sb[:], in_=bpc[:])
            io_sb = cpool.tile([BLK, TILE], bf16)
            nc.sync.dma_start(out=io_sb[:], in_=iota[:])
            xT_sb = cpool.tile([D, ROWS_PC], bf16)
            ix_sb = tpool.tile([BLK, W16], i16)
            sl0 = min(16, int(grp_nb[0])) * 8
            nc.sync.dma_start(out=ix_sb[:, :sl0], in_=ix16[:, :sl0])
            dl_lb = tpool.tile([BLK, NVIEWS], bf16)
            nc.sync.dma_start(out=dl_lb[:], in_=dl[:])
            dd_lb = tpool.tile([BLK, NB], bf16)
            nc.sync.dma_start(out=dd_lb[:], in_=dd[:])
            dl_sb = tpool.tile([BLK, NVIEWS], f32)
            nc.vector.tensor_copy(out=dl_sb[:], in_=dl_lb[:])
            dd_sb = tpool.tile([BLK, NB], f32)
            nc.vector.tensor_copy(out=dd_sb[:], in_=dd_lb[:])
            if int(grp_nb[0]) * 8 > sl0:
                nc.sync.dma_start(out=ix_sb[:, sl0:int(grp_nb[0]) * 8],
                                  in_=ix16[:, sl0:int(grp_nb[0]) * 8])
            for q in range(1, len(grp_slots)):
                a, b = int(grp_off[q]) * 8, (int(grp_off[q]) + int(grp_nb[q])) * 8
                nc.sync.dma_start(out=ix_sb[:, a:b], in_=ix16[:, a:b])
            nc.sync.dma_start(out=xT_sb[:], in_=xT[:])
            xsT_sb = cpool.tile([D, ROWS_PC], bf16)
            nc.sync.dma_start(out=xsT_sb[:], in_=xsT[:])

            qrr = [0]
            vcnt = [0]
            for q, sl in enumerate(grp_slots):
                gb0 = int(grp_off[q])
                gnb = int(grp_nb[q])
                g_sb = gpool.tile([BLK, gnb * D], bf16, tag="g")
                if "gather" not in ablate:
                    for (is_hi, b0, nb) in callplan[q]:
                        tbl = x_hi if is_hi else x_lo
                        nidx = nb * BLK
                        nc.gpsimd.dma_gather(
                            out_ap=g_sb[:, b0 * D:(b0 + nb) * D].rearrange(
                                "p (n e) -> p n e", e=D),
                            in_ap=tbl[:],
                            idxs_ap=ix_sb[:, (gb0 + b0) * 8:(gb0 + b0 + nb) * 8],
                            num_idxs=nidx,
                            num_idxs_reg=nidx,
                            elem_size=D,
                            queue_num=qrr[0] % N_SWDGE_QUEUES,
                            single_packet=False,
                        )
                        qrr[0] += 1
                og_sb = ogpool.tile([D, len(sl) * TILE], bf16, tag="og")
                zparts = {}
                for h in (0, 1):
                    for i in sl:
                        views = slot_views[i][h]
                        if not views or "segmm" in ablate:
                            continue
                        nv = len(views)
                        z_ps = zpool.tile([D, TILE], f32)
                        for jj, (gb, vcol) in enumerate(views):
                            lb = gb - gb0
                            d0, w = int(d0_v[vcol]), int(d1_v[vcol] - d0_v[vcol])
                            if "onehot" not in ablate:
                                oh = ohpool.tile([BLK, TILE], bf16, tag="oh")
                                nc.vector.tensor_scalar(
                                    out=oh[:, :w], in0=io_sb[:, :w],
                                    scalar1=dl_sb[:, vcol:vcol + 1],
                                    scalar2=dd_sb[:, gb:gb + 1],
                                    op0=eq, op1=mul,
                                )
                                rhs = oh[:, :w]
                            else:
                                rhs = io_sb[:, :w]
                            nc.tensor.matmul(
                                out=z_ps[:, d0:d0 + w],
                                lhsT=g_sb[:, lb * D:(lb + 1) * D],
                                rhs=rhs,
                                start=(jj == 0), stop=(jj == nv - 1),
                            )
                        z_sb = zsbpool.tile([D, TILE], bf16, tag="z")
                        nc.scalar.copy(out=z_sb[:], in_=z_ps[:])
                        zparts.setdefault(i, []).append(z_sb)
                if "epilogue" not in ablate:
                    for s, i in enumerate(sl):
                        o_ps = opool.tile([D, TILE], f32)
                        parts = zparts.get(i, [])
                        for z_sb in parts:
                            nc.tensor.matmul(out=o_ps[:], lhsT=c_sb[:],
                                             rhs=z_sb[:],
                                             start=(z_sb is parts[0]),
                                             stop=False)
                        nc.tensor.matmul(out=o_ps[:], lhsT=c_sb[:],
                                         rhs=xsT_sb[:, i * TILE:(i + 1) * TILE],
                                         start=(not parts), stop=False)
                        nc.tensor.matmul(out=o_ps[:], lhsT=b2_sb[:],
                                         rhs=xT_sb[:, i * TILE:(i + 1) * TILE],
                                         start=False, stop=True)
                        nc.scalar.activation(
                            out=og_sb[:, s * TILE:(s + 1) * TILE], in_=o_ps[:],
                            func=ident, bias=bp_sb[:, 0:1])
                if "epilogue" not in ablate:
                    col0 = sl[0] * TILE
                    nc.sync.dma_start(
                        out=out[:, col0:col0 + len(sl) * TILE], in_=og_sb[:])
    nc.compile()
    return nc


def _meta_key(meta):
    return (
        tuple(int(v) for v in meta["grp_nb"]),
        tuple(len(v) for v in meta["slot_views"]),
        int(meta["NVIEWS"]),
    )


def _get_nc(meta):
    key = _meta_key(meta)
    if key not in _NC_CACHE:
        _NC_CACHE[key] = _build_nc(meta)
    return _NC_CACHE[key]


def kernel(x, edge_index, Wc, bc, W0, Wt):
    global _LAST_RESULTS
    from concourse.bass_utils import run_bass_kernel_spmd

    x = np.asarray(x)
    n = x.shape[0]
    in_maps, meta = _host_prep(x, edge_index, Wc, bc, W0, Wt)
    nc = _get_nc(meta)
    res = run_bass_kernel_spmd(nc, in_maps, list(range(N_CORES)))
    _LAST_RESULTS = res
    out_full = np.empty((N_PAD, D), np.float32)
    for k in range(N_CORES):
        out_full[meta["node_ids"][k]] = np.asarray(
            res.results[k]["out"]).astype(np.float32).T
    return out_full[:n].astype(np.float32)


# revision 7
# speedup vs baseline: 1.3223x; 1.0140x over previous
"""DGCN on 8 Trainium2 NeuronCores, v3: shared-boundary max-profile packing.

Differences vs v2:
  - Within a (group, half) gather region, slots are packed back-to-back at
    their max-over-cores edge counts (no per-slot ceil-to-128); only each
    region is block-aligned. Blocks spanning a slot boundary get one
    matmul+one-hot per covered slot (per-view dl columns).
  - Octets balanced by total edge count (primary) then lo count.
  - Flexible group sizes, ending with a tiny group so the post-gather
    pipeline tail is short.
"""

import numpy as np

N_NODES = 50000
D = 128
N_CORES = 8
TILE = 128
BLK = 128
HALF = 32768
MAX_IDX_CALL = 1024  # per dma_gather call
N_SWDGE_QUEUES = 4
OFFLOAD_K = 0        # one-hot gpsimd offload disabled (head-of-line blocks gathers)

N_TILES = 392
TPC = N_TILES // N_CORES          # 49
GROUP_SIZES = [12, 12, 12, 12, 1]  # slots per group; tiny last = short tail
N_PAD = N_TILES * TILE
ROWS_PC = TPC * TILE

_NC_CACHE = {}
_LAST_RESULTS = None


def _host_prep(x, edge_index, Wc, bc, W0, Wt):
    import ml_dtypes

    bf16 = ml_dtypes.bfloat16
    n, d = x.shape
    src = np.asarray(edge_index[0], dtype=np.int64)
    dst = np.asarray(edge_index[1], dtype=np.int64)

    deg = (np.bincount(dst, minlength=N_PAD) + 1).astype(np.float32)
    dinv = (1.0 / np.sqrt(deg)).astype(np.float32)

    src_a = src
    dst_a = dst

    tile_g = dst_a // TILE
    ishi = (src_a >= HALF).astype(np.int64)
    order0 = np.lexsort((dst_a, ishi, tile_g))
    src_s = src_a[order0]
    dst_s = dst_a[order0]
    tile_s = tile_g[order0]

    counts = np.bincount(tile_s, minlength=N_TILES)
    starts = np.zeros(N_TILES + 1, np.int64)
    starts[1:] = np.cumsum(counts)
    lo_counts = np.bincount(tile_g[ishi == 0], minlength=N_TILES)
    hi_counts = counts - lo_counts

    # octet balancing: sort tiles by (lo, hi) DESCENDING and deal octet i
    # across the 8 cores as slot i; biggest slots first so the final (tiny)
    # group holds the smallest tiles. Then a pairwise same-core swap
    # hill-climb tightens sum(max lo + max hi) over octets.
    order_t = np.lexsort((hi_counts, lo_counts))[::-1]
    assign = order_t.reshape(TPC, N_CORES).copy()   # [slot, core] -> tile
    for _sweep in range(3):
        improved = 0
        alo = lo_counts[assign]
        ahi = hi_counts[assign]
        for i in range(TPC):
            for j in range(i + 1, TPC):
                base_cost = (alo[i].max() + ahi[i].max()
                             + alo[j].max() + ahi[j].max())
                for k in range(N_CORES):
                    alo[i, k], alo[j, k] = alo[j, k], alo[i, k]
                    ahi[i, k], ahi[j, k] = ahi[j, k], ahi[i, k]
                    new_cost = (alo[i].max() + ahi[i].max()
                                + alo[j].max() + ahi[j].max())
                    if new_cost < base_cost:
                        assign[i, k], assign[j, k] = assign[j, k], assign[i, k]
                        base_cost = new_cost
                        improved += 1
                    else:
                        alo[i, k], alo[j, k] = alo[j, k], alo[i, k]
                        ahi[i, k], ahi[j, k] = ahi[j, k], ahi[i, k]
        if improved == 0:
            break
    L_lo = lo_counts[assign].max(axis=1)            # [slot] max-profile lens
    L_hi = hi_counts[assign].max(axis=1)

    assert sum(GROUP_SIZES) == TPC
    grp_slots = []
    s0_ = 0
    for gs in GROUP_SIZES:
        grp_slots.append(list(range(s0_, s0_ + gs)))
        s0_ += gs

    # region/block/view layout
    NBASE = 0                    # running global block count
    callplan = []                # per group: (is_hi, local_b0, nb_blocks)
    grp_nb = []                  # blocks per group
    grp_off = []                 # global block offset per group
    slot_views_h = [([], []) for _ in range(TPC)]  # per half: (global_block, dl_col)
    # per-slot placement info for table building:
    place = {}                   # (slot, half) -> (region_pos, region_glb_b0)
    nview = 0
    for q, sl in enumerate(grp_slots):
        grp_off.append(NBASE)
        calls = []
        gb = 0                   # group-local block counter
        for h, L in ((0, L_lo), (1, L_hi)):
            pos = 0
            covered = []         # (slot, p0, p1) in region slot-positions
            for i in sl:
                li = int(L[i])
                place[(i, h)] = (pos, NBASE + gb)
                if li > 0:
                    covered.append((i, pos, pos + li))
                pos += li
            rblocks = -(-pos // BLK)
            # views: block b covers slot i iff ranges overlap
            for b in range(rblocks):
                lo_p, hi_p = b * BLK, (b + 1) * BLK
                for (i, p0, p1) in covered:
                    if p0 < hi_p and p1 > lo_p:
                        slot_views_h[i][h].append((NBASE + gb + b, nview))
                        nview += 1
            # calls (block-aligned, <=8 blocks each)
            b0 = gb
            nb = rblocks
            while nb > 0:
                take = min(nb, MAX_IDX_CALL // BLK)
                calls.append((h, b0, take))
                b0 += take
                nb -= take
            gb += rblocks
        grp_nb.append(gb)
        NBASE += gb
        callplan.append(calls)
    NB = NBASE
    NVIEWS = nview

    # tables
    idx_flat = np.zeros((N_CORES, NB * BLK), np.int32)
    dd_t = np.zeros((N_CORES, BLK, NB), np.float32)
    dl_t = np.full((N_CORES, BLK, NVIEWS), 1000.0, np.float32)
    for k in range(N_CORES):
        for i in range(TPC):
            g = assign[i, k]
            s0 = int(starts[g])
            clo, chi = int(lo_counts[g]), int(hi_counts[g])
            base = g * TILE
            for h, cnt, shift, pos0 in ((0, clo, 0, s0), (1, chi, HALF, s0 + clo)):
                if cnt == 0:
                    continue
                rpos, rgb0 = place[(i, h)]
                e0 = rgb0 * BLK + rpos
                idx_flat[k, e0:e0 + cnt] = src_s[pos0:pos0 + cnt] - shift
                # dd_t is [BLK, NB] with flat pos j -> [j%BLK, j//BLK]
                jj = np.arange(e0, e0 + cnt)
                dd_t[k][jj % BLK, jj // BLK] = dinv[dst_s[pos0:pos0 + cnt]]
        # dl per view (pass B below; geometry shared with pass A)

    def _view_rows(k):
        for i in range(TPC):
            g = assign[i, k]
            s0 = int(starts[g])
            clo, chi = int(lo_counts[g]), int(hi_counts[g])
            base = g * TILE
            for h in (0, 1):
                cnt = clo if h == 0 else chi
                pos0 = s0 if h == 0 else s0 + clo
                rpos, rgb0 = place.get((i, h), (None, None))
                if rpos is None:
                    continue
                li = int((L_lo if h == 0 else L_hi)[i])
                b_lo = rgb0 * BLK + rpos
                b_hi = b_lo + li
                for jj, (gb, vcol) in enumerate(slot_views_h[i][h]):
                    blk_lo, blk_hi = gb * BLK, (gb + 1) * BLK
                    if not (b_lo < blk_hi and b_hi > blk_lo):
                        continue
                    p_start = max(b_lo, blk_lo)
                    p_end = min(b_hi, blk_hi)
                    r0 = p_start - blk_lo
                    eo0 = p_start - b_lo
                    m = min(cnt - eo0, p_end - p_start)
                    yield (vcol, jj, r0, m, pos0 + eo0, base)

    d0_v = np.full(NVIEWS, TILE, np.int64)
    d1_v = np.zeros(NVIEWS, np.int64)
    first_v = np.zeros(NVIEWS, bool)
    for k in range(N_CORES):
        for (vcol, jj, r0, m, p0, base) in _view_rows(k):
            if jj == 0:
                first_v[vcol] = True
            if m > 0:
                dv = dst_s[p0:p0 + m] - base
                d0_v[vcol] = min(d0_v[vcol], int(dv.min()))
                d1_v[vcol] = max(d1_v[vcol], int(dv.max()) + 1)
    d0_v[first_v] = 0
    d1_v[first_v] = TILE
    bad = d1_v <= d0_v
    d0_v[bad] = 0
    d1_v[bad] = 1

    W16 = NB * (BLK // 16)
    for k in range(N_CORES):
        for (vcol, jj, r0, m, p0, base) in _view_rows(k):
            if m > 0:
                dl_t[k][r0:r0 + m, vcol] = (
                    dst_s[p0:p0 + m] - base - d0_v[vcol]).astype(np.float32)

    ix16 = np.empty((N_CORES, BLK, W16), np.int16)
    for k in range(N_CORES):
        v = idx_flat[k].astype(np.int16).reshape(W16, 16).T
        ix16[k] = np.tile(v, (8, 1))

    Wc64 = np.asarray(Wc, np.float64)
    Wt64 = np.asarray(Wt, np.float64)
    W064 = np.asarray(W0, np.float64)
    bc64 = np.asarray(bc, np.float64)
    B1 = np.eye(d) + Wt64
    C = (Wc64 @ B1).astype(np.float32)
    B2 = (W064 - Wt64).astype(np.float32)
    bp = (bc64 @ B1).astype(np.float32)

    x_pad = np.zeros((N_PAD, d), np.float32)
    x_pad[:n] = np.asarray(x, np.float32)
    xs = (x_pad * dinv[:, None]).astype(bf16)
    xself = x_pad * (dinv * dinv)[:, None]

    iota = np.broadcast_to(np.arange(TILE, dtype=np.float32),
                           (BLK, TILE)).astype(bf16)

    node_ids = np.empty((N_CORES, ROWS_PC), np.int64)
    for k in range(N_CORES):
        for i in range(TPC):
            g = assign[i, k]
            node_ids[k, i * TILE:(i + 1) * TILE] = np.arange(
                g * TILE, (g + 1) * TILE)

    in_maps = []
    for k in range(N_CORES):
        m = {
            "x_lo": xs[:HALF],
            "x_hi": xs[HALF:],
            "xT": np.ascontiguousarray(x_pad[node_ids[k]].T.astype(bf16)),
            "xsT": np.ascontiguousarray(xself[node_ids[k]].T.astype(bf16)),
            "ix16": ix16[k],
            "dl": dl_t[k].astype(bf16),
            "dd": dd_t[k].astype(bf16),
            "cw": C.astype(bf16),
            "b2w": B2.astype(bf16),
            "bpc": bp.reshape(d, 1),
            "iota": np.ascontiguousarray(iota),
        }
        in_maps.append(m)

    meta = dict(
        grp_slots=grp_slots, grp_nb=grp_nb, grp_off=grp_off,
        callplan=callplan, slot_views=slot_views_h, NB=NB, NVIEWS=NVIEWS,
        d0_v=d0_v, d1_v=d1_v,
        W16=W16, hi_rows=N_PAD - HALF, node_ids=node_ids,
    )
    return in_maps, meta


def _build_nc(meta, ablate=()):
    import concourse.bacc as bacc
    import concourse.mybir as mybir
    import concourse.tile as tile
    from concourse import library_config

    f32 = mybir.dt.float32
    bf16 = mybir.dt.bfloat16
    i16 = mybir.dt.int16
    eq, mul = mybir.AluOpType.is_equal, mybir.AluOpType.mult
    ident = mybir.ActivationFunctionType.Identity

    grp_slots, grp_nb, grp_off = meta["grp_slots"], meta["grp_nb"], meta["grp_off"]
    callplan, slot_views = meta["callplan"], meta["slot_views"]
    NB, NVIEWS, W16 = meta["NB"], meta["NVIEWS"], meta["W16"]
    d0_v, d1_v = meta["d0_v"], meta["d1_v"]

    nc = bacc.Bacc(
        "TRN2",
        target_bir_lowering=False,
        debug=False,
        num_devices=N_CORES,
        num_swdge_queues=N_SWDGE_QUEUES,
    )
    x_lo = nc.declare_dram_parameter("x_lo", [HALF, D], bf16, isOutput=False)
    x_hi = nc.declare_dram_parameter("x_hi", [meta["hi_rows"], D], bf16,
                                     isOutput=False)
    xT = nc.declare_dram_parameter("xT", [D, ROWS_PC], bf16, isOutput=False)
    xsT = nc.declare_dram_parameter("xsT", [D, ROWS_PC], bf16, isOutput=False)
    ix16 = nc.declare_dram_parameter("ix16", [BLK, W16], i16, isOutput=False)
    dl = nc.declare_dram_parameter("dl", [BLK, NVIEWS], bf16, isOutput=False)
    dd = nc.declare_dram_parameter("dd", [BLK, NB], bf16, isOutput=False)
    cw = nc.declare_dram_parameter("cw", [D, D], bf16, isOutput=False)
    b2w = nc.declare_dram_parameter("b2w", [D, D], bf16, isOutput=False)
    bpc = nc.declare_dram_parameter("bpc", [D, 1], f32, isOutput=False)
    iota = nc.declare_dram_parameter("iota", [BLK, TILE], bf16, isOutput=False)
    out = nc.declare_dram_parameter("out", [D, ROWS_PC], bf16, isOutput=True)

    with tile.TileContext(nc) as tc:
        with (
            tc.tile_pool(name="const", bufs=1) as cpool,
            tc.tile_pool(name="tbl", bufs=1) as tpool,
            tc.tile_pool(name="gather", bufs=2) as gpool,
            tc.tile_pool(name="oh", bufs=16) as ohpool,
            tc.tile_pool(name="z", bufs=24) as zsbpool,
            tc.tile_pool(name="og", bufs=2) as ogpool,
            tc.tile_pool(name="zps", bufs=6, space="PSUM") as zpool,
            tc.tile_pool(name="ops", bufs=2, space="PSUM") as opool,
        ):
            nc.gpsimd.load_library(library_config.mlp)
            c_sb = cpool.tile([D, D], bf16)
            nc.sync.dma_start(out=c_sb[:], in_=cw[:])
            b2_sb = cpool.tile([D, D], bf16)
            nc.sync.dma_start(out=b2_sb[:], in_=b2w[:])
            bp_sb = cpool.tile([D, 1], f32)
            nc.sync.dma_start(out=bp_sb[:], in_=bpc[:])
            io_sb = cpool.tile([BLK, TILE], bf16)
            nc.sync.dma_start(out=io_sb[:], in_=iota[:])
            xT_sb = cpool.tile([D, ROWS_PC], bf16)
            ix_sb = tpool.tile([BLK, W16], i16)
            sl0 = min(16, int(grp_nb[0])) * 8
            nc.scalar.dma_start(out=ix_sb[:, :sl0], in_=ix16[:, :sl0])
            dl_lb = tpool.tile([BLK, NVIEWS], bf16)
            nc.scalar.dma_start(out=dl_lb[:], in_=dl[:])
            dd_lb = tpool.tile([BLK, NB], bf16)
            nc.scalar.dma_start(out=dd_lb[:], in_=dd[:])
            dl_sb = tpool.tile([BLK, NVIEWS], f32)
            nc.vector.tensor_copy(out=dl_sb[:], in_=dl_lb[:])
            dd_sb = tpool.tile([BLK, NB], f32)
            nc.vector.tensor_copy(out=dd_sb[:], in_=dd_lb[:])
            if int(grp_nb[0]) * 8 > sl0:
                nc.sync.dma_start(out=ix_sb[:, sl0:int(grp_nb[0]) * 8],
                                  in_=ix16[:, sl0:int(grp_nb[0]) * 8])
            for q in range(1, len(grp_slots)):
                a, b = int(grp_off[q]) * 8, (int(grp_off[q]) + int(grp_nb[q])) * 8
                nc.sync.dma_start(out=ix_sb[:, a:b], in_=ix16[:, a:b])
            nc.sync.dma_start(out=xT_sb[:], in_=xT[:])
            xsT_sb = cpool.tile([D, ROWS_PC], bf16)
            nc.sync.dma_start(out=xsT_sb[:], in_=xsT[:])

            qrr = [0]
            vcnt = [0]
            for q, sl in enumerate(grp_slots):
                gb0 = int(grp_off[q])
                gnb = int(grp_nb[q])
                g_sb = gpool.tile([BLK, gnb * D], bf16, tag="g")
                if "gather" not in ablate:
                    for (is_hi, b0, nb) in callplan[q]:
                        tbl = x_hi if is_hi else x_lo
                        nidx = nb * BLK
                        nc.gpsimd.dma_gather(
                            out_ap=g_sb[:, b0 * D:(b0 + nb) * D].rearrange(
                                "p (n e) -> p n e", e=D),
                            in_ap=tbl[:],
                            idxs_ap=ix_sb[:, (gb0 + b0) * 8:(gb0 + b0 + nb) * 8],
                            num_idxs=nidx,
                            num_idxs_reg=nidx,
                            elem_size=D,
                            queue_num=qrr[0] % N_SWDGE_QUEUES,
                            single_packet=False,
                        )
                        qrr[0] += 1
                og_sb = ogpool.tile([D, len(sl) * TILE], bf16, tag="og")
                zparts = {}
                for h in (0, 1):
                    for i in sl:
                        views = slot_views[i][h]
                        if not views or "segmm" in ablate:
                            continue
                        nv = len(views)
                        z_ps = zpool.tile([D, TILE], f32)
                        for jj, (gb, vcol) in enumerate(views):
                            lb = gb - gb0
                            d0, w = int(d0_v[vcol]), int(d1_v[vcol] - d0_v[vcol])
                            if "onehot" not in ablate:
                                oh = ohpool.tile([BLK, TILE], bf16, tag="oh")
                                nc.vector.tensor_scalar(
                                    out=oh[:, :w], in0=io_sb[:, :w],
                                    scalar1=dl_sb[:, vcol:vcol + 1],
                                    scalar2=dd_sb[:, gb:gb + 1],
                                    op0=eq, op1=mul,
                                )
                                rhs = oh[:, :w]
                            else:
                                rhs = io_sb[:, :w]
                            nc.tensor.matmul(
                                out=z_ps[:, d0:d0 + w],
                                lhsT=g_sb[:, lb * D:(lb + 1) * D],
                                rhs=rhs,
                                start=(jj == 0), stop=(jj == nv - 1),
                            )
                        z_sb = zsbpool.tile([D, TILE], bf16, tag="z")
                        nc.scalar.copy(out=z_sb[:], in_=z_ps[:])
                        zparts.setdefault(i, []).append(z_sb)
                if "epilogue" not in ablate:
                    for s, i in enumerate(sl):
                        o_ps = opool.tile([D, TILE], f32)
                        parts = zparts.get(i, [])
                        for z_sb in parts:
                            nc.tensor.matmul(out=o_ps[:], lhsT=c_sb[:],
                                             rhs=z_sb[:],
                                             start=(z_sb is parts[0]),
                                             stop=False)
                        nc.tensor.matmul(out=o_ps[:], lhsT=c_sb[:],
                                         rhs=xsT_sb[:, i * TILE:(i + 1) * TILE],
                                         start=(not parts), stop=False)
                        nc.tensor.matmul(out=o_ps[:], lhsT=b2_sb[:],
                                         rhs=xT_sb[:, i * TILE:(i + 1) * TILE],
                                         start=False, stop=True)
                        nc.scalar.activation(
                            out=og_sb[:, s * TILE:(s + 1) * TILE], in_=o_ps[:],
                            func=ident, bias=bp_sb[:, 0:1])
                if "epilogue" not in ablate:
                    col0 = sl[0] * TILE
                    nc.sync.dma_start(
                        out=out[:, col0:col0 + len(sl) * TILE], in_=og_sb[:])
    nc.compile()
    return nc


def _meta_key(meta):
    return (
        tuple(int(v) for v in meta["grp_nb"]),
        tuple(len(v) for v in meta["slot_views"]),
        int(meta["NVIEWS"]),
    )


def _get_nc(meta):
    key = _meta_key(meta)
    if key not in _NC_CACHE:
        _NC_CACHE[key] = _build_nc(meta)
    return _NC_CACHE[key]


def kernel(x, edge_index, Wc, bc, W0, Wt):
    global _LAST_RESULTS
    from concourse.bass_utils import run_bass_kernel_spmd

    x = np.asarray(x)
    n = x.shape[0]
    in_maps, meta = _host_prep(x, edge_index, Wc, bc, W0, Wt)
    nc = _get_nc(meta)
    res = run_bass_kernel_spmd(nc, in_maps, list(range(N_CORES)))
    _LAST_RESULTS = res
    out_full = np.empty((N_PAD, D), np.float32)
    for k in range(N_CORES):
        out_full[meta["node_ids"][k]] = np.asarray(
            res.results[k]["out"]).astype(np.float32).T
    return out_full[:n].astype(np.float32)


# revision 8
# speedup vs baseline: 1.3260x; 1.0028x over previous
"""DGCN on 8 Trainium2 NeuronCores, v3: shared-boundary max-profile packing.

Differences vs v2:
  - Within a (group, half) gather region, slots are packed back-to-back at
    their max-over-cores edge counts (no per-slot ceil-to-128); only each
    region is block-aligned. Blocks spanning a slot boundary get one
    matmul+one-hot per covered slot (per-view dl columns).
  - Octets balanced by total edge count (primary) then lo count.
  - Flexible group sizes, ending with a tiny group so the post-gather
    pipeline tail is short.
"""

import numpy as np

N_NODES = 50000
D = 128
N_CORES = 8
TILE = 128
BLK = 128
HALF = 32768
MAX_IDX_CALL = 1024  # per dma_gather call
N_SWDGE_QUEUES = 4
OFFLOAD_K = 0        # one-hot gpsimd offload disabled (head-of-line blocks gathers)

N_TILES = 392
TPC = N_TILES // N_CORES          # 49
GROUP_SIZES = [12, 12, 12, 12, 1]  # slots per group; tiny last = short tail
N_PAD = N_TILES * TILE
ROWS_PC = TPC * TILE

_NC_CACHE = {}
_LAST_RESULTS = None


def _host_prep(x, edge_index, Wc, bc, W0, Wt):
    import ml_dtypes

    bf16 = ml_dtypes.bfloat16
    n, d = x.shape
    src = np.asarray(edge_index[0], dtype=np.int64)
    dst = np.asarray(edge_index[1], dtype=np.int64)

    deg = (np.bincount(dst, minlength=N_PAD) + 1).astype(np.float32)
    dinv = (1.0 / np.sqrt(deg)).astype(np.float32)

    src_a = src
    dst_a = dst

    tile_g = dst_a // TILE
    ishi = (src_a >= HALF).astype(np.int64)
    order0 = np.lexsort((dst_a, ishi, tile_g))
    src_s = src_a[order0]
    dst_s = dst_a[order0]
    tile_s = tile_g[order0]

    counts = np.bincount(tile_s, minlength=N_TILES)
    starts = np.zeros(N_TILES + 1, np.int64)
    starts[1:] = np.cumsum(counts)
    lo_counts = np.bincount(tile_g[ishi == 0], minlength=N_TILES)
    hi_counts = counts - lo_counts

    # octet balancing: sort tiles by (lo, hi) DESCENDING and deal octet i
    # across the 8 cores as slot i; biggest slots first so the final (tiny)
    # group holds the smallest tiles. Then a pairwise same-core swap
    # hill-climb tightens sum(max lo + max hi) over octets.
    order_t = np.lexsort((hi_counts, lo_counts))[::-1]
    assign = order_t.reshape(TPC, N_CORES).copy()   # [slot, core] -> tile
    for _sweep in range(3):
        improved = 0
        alo = lo_counts[assign]
        ahi = hi_counts[assign]
        for i in range(TPC):
            for j in range(i + 1, TPC):
                base_cost = (alo[i].max() + ahi[i].max()
                             + alo[j].max() + ahi[j].max())
                for k in range(N_CORES):
                    alo[i, k], alo[j, k] = alo[j, k], alo[i, k]
                    ahi[i, k], ahi[j, k] = ahi[j, k], ahi[i, k]
                    new_cost = (alo[i].max() + ahi[i].max()
                                + alo[j].max() + ahi[j].max())
                    if new_cost < base_cost:
                        assign[i, k], assign[j, k] = assign[j, k], assign[i, k]
                        base_cost = new_cost
                        improved += 1
                    else:
                        alo[i, k], alo[j, k] = alo[j, k], alo[i, k]
                        ahi[i, k], ahi[j, k] = ahi[j, k], ahi[i, k]
        if improved == 0:
            break
    L_lo = lo_counts[assign].max(axis=1)            # [slot] max-profile lens
    L_hi = hi_counts[assign].max(axis=1)

    assert sum(GROUP_SIZES) == TPC
    grp_slots = []
    s0_ = 0
    for gs in GROUP_SIZES:
        grp_slots.append(list(range(s0_, s0_ + gs)))
        s0_ += gs

    # region/block/view layout
    NBASE = 0                    # running global block count
    callplan = []                # per group: (is_hi, local_b0, nb_blocks)
    grp_nb = []                  # blocks per group
    grp_off = []                 # global block offset per group
    slot_views_h = [([], []) for _ in range(TPC)]  # per half: (global_block, dl_col)
    # per-slot placement info for table building:
    place = {}                   # (slot, half) -> (region_pos, region_glb_b0)
    nview = 0
    for q, sl in enumerate(grp_slots):
        grp_off.append(NBASE)
        calls = []
        gb = 0                   # group-local block counter
        for h, L in ((0, L_lo), (1, L_hi)):
            pos = 0
            covered = []         # (slot, p0, p1) in region slot-positions
            for i in sl:
                li = int(L[i])
                place[(i, h)] = (pos, NBASE + gb)
                if li > 0:
                    covered.append((i, pos, pos + li))
                pos += li
            rblocks = -(-pos // BLK)
            # views: block b covers slot i iff ranges overlap
            for b in range(rblocks):
                lo_p, hi_p = b * BLK, (b + 1) * BLK
                for (i, p0, p1) in covered:
                    if p0 < hi_p and p1 > lo_p:
                        slot_views_h[i][h].append((NBASE + gb + b, nview))
                        nview += 1
            # calls (block-aligned, <=8 blocks each)
            b0 = gb
            nb = rblocks
            while nb > 0:
                take = min(nb, MAX_IDX_CALL // BLK)
                calls.append((h, b0, take))
                b0 += take
                nb -= take
            gb += rblocks
        grp_nb.append(gb)
        NBASE += gb
        callplan.append(calls)
    NB = NBASE
    NVIEWS = nview

    # tables
    idx_flat = np.zeros((N_CORES, NB * BLK), np.int32)
    dd_t = np.zeros((N_CORES, BLK, NB), np.float32)
    dl_t = np.full((N_CORES, BLK, NVIEWS), 1000.0, np.float32)
    for k in range(N_CORES):
        for i in range(TPC):
            g = assign[i, k]
            s0 = int(starts[g])
            clo, chi = int(lo_counts[g]), int(hi_counts[g])
            base = g * TILE
            for h, cnt, shift, pos0 in ((0, clo, 0, s0), (1, chi, HALF, s0 + clo)):
                if cnt == 0:
                    continue
                rpos, rgb0 = place[(i, h)]
                e0 = rgb0 * BLK + rpos
                idx_flat[k, e0:e0 + cnt] = src_s[pos0:pos0 + cnt] - shift
                # dd_t is [BLK, NB] with flat pos j -> [j%BLK, j//BLK]
                jj = np.arange(e0, e0 + cnt)
                dd_t[k][jj % BLK, jj // BLK] = dinv[dst_s[pos0:pos0 + cnt]]
        # dl per view (pass B below; geometry shared with pass A)

    def _view_rows(k):
        for i in range(TPC):
            g = assign[i, k]
            s0 = int(starts[g])
            clo, chi = int(lo_counts[g]), int(hi_counts[g])
            base = g * TILE
            for h in (0, 1):
                cnt = clo if h == 0 else chi
                pos0 = s0 if h == 0 else s0 + clo
                rpos, rgb0 = place.get((i, h), (None, None))
                if rpos is None:
                    continue
                li = int((L_lo if h == 0 else L_hi)[i])
                b_lo = rgb0 * BLK + rpos
                b_hi = b_lo + li
                for jj, (gb, vcol) in enumerate(slot_views_h[i][h]):
                    blk_lo, blk_hi = gb * BLK, (gb + 1) * BLK
                    if not (b_lo < blk_hi and b_hi > blk_lo):
                        continue
                    p_start = max(b_lo, blk_lo)
                    p_end = min(b_hi, blk_hi)
                    r0 = p_start - blk_lo
                    eo0 = p_start - b_lo
                    m = min(cnt - eo0, p_end - p_start)
                    yield (vcol, jj, r0, m, pos0 + eo0, base)

    d0_v = np.full(NVIEWS, TILE, np.int64)
    d1_v = np.zeros(NVIEWS, np.int64)
    first_v = np.zeros(NVIEWS, bool)
    for k in range(N_CORES):
        for (vcol, jj, r0, m, p0, base) in _view_rows(k):
            if jj == 0:
                first_v[vcol] = True
            if m > 0:
                dv = dst_s[p0:p0 + m] - base
                d0_v[vcol] = min(d0_v[vcol], int(dv.min()))
                d1_v[vcol] = max(d1_v[vcol], int(dv.max()) + 1)
    d0_v[first_v] = 0
    d1_v[first_v] = TILE
    bad = d1_v <= d0_v
    d0_v[bad] = 0
    d1_v[bad] = 1

    W16 = NB * (BLK // 16)
    for k in range(N_CORES):
        for (vcol, jj, r0, m, p0, base) in _view_rows(k):
            if m > 0:
                dl_t[k][r0:r0 + m, vcol] = (
                    dst_s[p0:p0 + m] - base - d0_v[vcol]).astype(np.float32)

    ix16 = np.empty((N_CORES, BLK, W16), np.int16)
    for k in range(N_CORES):
        v = idx_flat[k].astype(np.int16).reshape(W16, 16).T
        ix16[k] = np.tile(v, (8, 1))

    Wc64 = np.asarray(Wc, np.float64)
    Wt64 = np.asarray(Wt, np.float64)
    W064 = np.asarray(W0, np.float64)
    bc64 = np.asarray(bc, np.float64)
    B1 = np.eye(d) + Wt64
    C = (Wc64 @ B1).astype(np.float32)
    B2 = (W064 - Wt64).astype(np.float32)
    bp = (bc64 @ B1).astype(np.float32)

    x_pad = np.zeros((N_PAD, d), np.float32)
    x_pad[:n] = np.asarray(x, np.float32)
    xs = (x_pad * dinv[:, None]).astype(bf16)
    xself = x_pad * (dinv * dinv)[:, None]

    iota = np.broadcast_to(np.arange(TILE, dtype=np.float32),
                           (BLK, TILE)).astype(bf16)

    node_ids = np.empty((N_CORES, ROWS_PC), np.int64)
    for k in range(N_CORES):
        for i in range(TPC):
            g = assign[i, k]
            node_ids[k, i * TILE:(i + 1) * TILE] = np.arange(
                g * TILE, (g + 1) * TILE)

    in_maps = []
    for k in range(N_CORES):
        m = {
            "x_lo": xs[:HALF],
            "x_hi": xs[HALF:],
            "xT": np.ascontiguousarray(x_pad[node_ids[k]].T.astype(bf16)),
            "xsT": np.ascontiguousarray(xself[node_ids[k]].T.astype(bf16)),
            "ix16": ix16[k],
            "dl": dl_t[k].astype(bf16),
            "dd": dd_t[k].astype(bf16),
            "cw": C.astype(bf16),
            "b2w": B2.astype(bf16),
            "bpc": bp.reshape(d, 1),
            "iota": np.ascontiguousarray(iota),
        }
        in_maps.append(m)

    meta = dict(
        grp_slots=grp_slots, grp_nb=grp_nb, grp_off=grp_off,
        callplan=callplan, slot_views=slot_views_h, NB=NB, NVIEWS=NVIEWS,
        d0_v=d0_v, d1_v=d1_v,
        W16=W16, hi_rows=N_PAD - HALF, node_ids=node_ids,
    )
    return in_maps, meta


def _build_nc(meta, ablate=()):
    import concourse.bacc as bacc
    import concourse.mybir as mybir
    import concourse.tile as tile
    from concourse import library_config

    f32 = mybir.dt.float32
    bf16 = mybir.dt.bfloat16
    i16 = mybir.dt.int16
    eq, mul = mybir.AluOpType.is_equal, mybir.AluOpType.mult
    ident = mybir.ActivationFunctionType.Identity

    grp_slots, grp_nb, grp_off = meta["grp_slots"], meta["grp_nb"], meta["grp_off"]
    callplan, slot_views = meta["callplan"], meta["slot_views"]
    NB, NVIEWS, W16 = meta["NB"], meta["NVIEWS"], meta["W16"]
    d0_v, d1_v = meta["d0_v"], meta["d1_v"]

    nc = bacc.Bacc(
        "TRN2",
        target_bir_lowering=False,
        debug=False,
        num_devices=N_CORES,
        num_swdge_queues=N_SWDGE_QUEUES,
    )
    x_lo = nc.declare_dram_parameter("x_lo", [HALF, D], bf16, isOutput=False)
    x_hi = nc.declare_dram_parameter("x_hi", [meta["hi_rows"], D], bf16,
                                     isOutput=False)
    xT = nc.declare_dram_parameter("xT", [D, ROWS_PC], bf16, isOutput=False)
    xsT = nc.declare_dram_parameter("xsT", [D, ROWS_PC], bf16, isOutput=False)
    ix16 = nc.declare_dram_parameter("ix16", [BLK, W16], i16, isOutput=False)
    dl = nc.declare_dram_parameter("dl", [BLK, NVIEWS], bf16, isOutput=False)
    dd = nc.declare_dram_parameter("dd", [BLK, NB], bf16, isOutput=False)
    cw = nc.declare_dram_parameter("cw", [D, D], bf16, isOutput=False)
    b2w = nc.declare_dram_parameter("b2w", [D, D], bf16, isOutput=False)
    bpc = nc.declare_dram_parameter("bpc", [D, 1], f32, isOutput=False)
    iota = nc.declare_dram_parameter("iota", [BLK, TILE], bf16, isOutput=False)
    out = nc.declare_dram_parameter("out", [D, ROWS_PC], bf16, isOutput=True)

    with tile.TileContext(nc) as tc:
        with (
            tc.tile_pool(name="const", bufs=1) as cpool,
            tc.tile_pool(name="tbl", bufs=1) as tpool,
            tc.tile_pool(name="gather", bufs=2) as gpool,
            tc.tile_pool(name="oh", bufs=16) as ohpool,
            tc.tile_pool(name="z", bufs=24) as zsbpool,
            tc.tile_pool(name="og", bufs=2) as ogpool,
            tc.tile_pool(name="zps", bufs=6, space="PSUM") as zpool,
            tc.tile_pool(name="ops", bufs=2, space="PSUM") as opool,
        ):
            nc.gpsimd.load_library(library_config.mlp)
            ix_sb = tpool.tile([BLK, W16], i16)
            sl0 = min(16, int(grp_nb[0])) * 8
            nc.sync.dma_start(out=ix_sb[:, :sl0], in_=ix16[:, :sl0])
            c_sb = cpool.tile([D, D], bf16)
            nc.sync.dma_start(out=c_sb[:], in_=cw[:])
            b2_sb = cpool.tile([D, D], bf16)
            nc.sync.dma_start(out=b2_sb[:], in_=b2w[:])
            bp_sb = cpool.tile([D, 1], f32)
            nc.sync.dma_start(out=bp_sb[:], in_=bpc[:])
            io_sb = cpool.tile([BLK, TILE], bf16)
            nc.sync.dma_start(out=io_sb[:], in_=iota[:])
            xT_sb = cpool.tile([D, ROWS_PC], bf16)
            dl_lb = tpool.tile([BLK, NVIEWS], bf16)
            nc.scalar.dma_start(out=dl_lb[:], in_=dl[:])
            dd_lb = tpool.tile([BLK, NB], bf16)
            nc.scalar.dma_start(out=dd_lb[:], in_=dd[:])
            dl_sb = tpool.tile([BLK, NVIEWS], f32)
            nc.vector.tensor_copy(out=dl_sb[:], in_=dl_lb[:])
            dd_sb = tpool.tile([BLK, NB], f32)
            nc.vector.tensor_copy(out=dd_sb[:], in_=dd_lb[:])
            if int(grp_nb[0]) * 8 > sl0:
                nc.sync.dma_start(out=ix_sb[:, sl0:int(grp_nb[0]) * 8],
                                  in_=ix16[:, sl0:int(grp_nb[0]) * 8])
            for q in range(1, len(grp_slots)):
                a, b = int(grp_off[q]) * 8, (int(grp_off[q]) + int(grp_nb[q])) * 8
                nc.sync.dma_start(out=ix_sb[:, a:b], in_=ix16[:, a:b])
            nc.sync.dma_start(out=xT_sb[:], in_=xT[:])
            xsT_sb = cpool.tile([D, ROWS_PC], bf16)
            nc.sync.dma_start(out=xsT_sb[:], in_=xsT[:])

            qrr = [0]
            vcnt = [0]
            for q, sl in enumerate(grp_slots):
                gb0 = int(grp_off[q])
                gnb = int(grp_nb[q])
                g_sb = gpool.tile([BLK, gnb * D], bf16, tag="g")
                if "gather" not in ablate:
                    for (is_hi, b0, nb) in callplan[q]:
                        tbl = x_hi if is_hi else x_lo
                        nidx = nb * BLK
                        nc.gpsimd.dma_gather(
                            out_ap=g_sb[:, b0 * D:(b0 + nb) * D].rearrange(
                                "p (n e) -> p n e", e=D),
                            in_ap=tbl[:],
                            idxs_ap=ix_sb[:, (gb0 + b0) * 8:(gb0 + b0 + nb) * 8],
                            num_idxs=nidx,
                            num_idxs_reg=nidx,
                            elem_size=D,
                            queue_num=qrr[0] % N_SWDGE_QUEUES,
                            single_packet=False,
                        )
                        qrr[0] += 1
                og_sb = ogpool.tile([D, len(sl) * TILE], bf16, tag="og")
                zparts = {}
                for h in (0, 1):
                    for i in sl:
                        views = slot_views[i][h]
                        if not views or "segmm" in ablate:
                            continue
                        nv = len(views)
                        z_ps = zpool.tile([D, TILE], f32)
                        for jj, (gb, vcol) in enumerate(views):
                            lb = gb - gb0
                            d0, w = int(d0_v[vcol]), int(d1_v[vcol] - d0_v[vcol])
                            if "onehot" not in ablate:
                                oh = ohpool.tile([BLK, TILE], bf16, tag="oh")
                                nc.vector.tensor_scalar(
                                    out=oh[:, :w], in0=io_sb[:, :w],
                                    scalar1=dl_sb[:, vcol:vcol + 1],
                                    scalar2=dd_sb[:, gb:gb + 1],
                                    op0=eq, op1=mul,
                                )
                                rhs = oh[:, :w]
                            else:
                                rhs = io_sb[:, :w]
                            nc.tensor.matmul(
                                out=z_ps[:, d0:d0 + w],
                                lhsT=g_sb[:, lb * D:(lb + 1) * D],
                                rhs=rhs,
                                start=(jj == 0), stop=(jj == nv - 1),
                            )
                        z_sb = zsbpool.tile([D, TILE], bf16, tag="z")
                        nc.scalar.copy(out=z_sb[:], in_=z_ps[:])
                        zparts.setdefault(i, []).append(z_sb)
                if "epilogue" not in ablate:
                    for s, i in enumerate(sl):
                        o_ps = opool.tile([D, TILE], f32)
                        parts = zparts.get(i, [])
                        for z_sb in parts:
                            nc.tensor.matmul(out=o_ps[:], lhsT=c_sb[:],
                                             rhs=z_sb[:],
                                             start=(z_sb is parts[0]),
                                             stop=False)
                        nc.tensor.matmul(out=o_ps[:], lhsT=c_sb[:],
                                         rhs=xsT_sb[:, i * TILE:(i + 1) * TILE],
                                         start=(not parts), stop=False)
                        nc.tensor.matmul(out=o_ps[:], lhsT=b2_sb[:],
                                         rhs=xT_sb[:, i * TILE:(i + 1) * TILE],
                                         start=False, stop=True)
                        nc.scalar.activation(
                            out=og_sb[:, s * TILE:(s + 1) * TILE], in_=o_ps[:],
                            func=ident, bias=bp_sb[:, 0:1])
                if "epilogue" not in ablate:
                    col0 = sl[0] * TILE
                    nc.sync.dma_start(
                        out=out[:, col0:col0 + len(sl) * TILE], in_=og_sb[:])
    nc.compile()
    return nc


def _meta_key(meta):
    return (
        tuple(int(v) for v in meta["grp_nb"]),
        tuple(len(v) for v in meta["slot_views"]),
        int(meta["NVIEWS"]),
    )


def _get_nc(meta):
    key = _meta_key(meta)
    if key not in _NC_CACHE:
        _NC_CACHE[key] = _build_nc(meta)
    return _NC_CACHE[key]


def kernel(x, edge_index, Wc, bc, W0, Wt):
    global _LAST_RESULTS
    from concourse.bass_utils import run_bass_kernel_spmd

    x = np.asarray(x)
    n = x.shape[0]
    in_maps, meta = _host_prep(x, edge_index, Wc, bc, W0, Wt)
    nc = _get_nc(meta)
    res = run_bass_kernel_spmd(nc, in_maps, list(range(N_CORES)))
    _LAST_RESULTS = res
    out_full = np.empty((N_PAD, D), np.float32)
    for k in range(N_CORES):
        out_full[meta["node_ids"][k]] = np.asarray(
            res.results[k]["out"]).astype(np.float32).T
    return out_full[:n].astype(np.float32)
